# revision 1
# baseline (speedup 1.0000x reference)
"""CellSpatialNet (4-layer NNConv GNN) on 8 trn2 NeuronCores.

Strategy: shard nodes+edges by dst across 8 cores (2560 nodes = 2 graphs/core).
Host folds the EdgeNN into 3 per-type tables G0/G1/G2 [36, d] so that
  W[e] = relu(ef0*G0[t_e] + ef1*G1[t_e] + G2[t_e])
becomes ONE PE matmul per 128-edge tile with a host-built "scaled one-hot"
stationary operand [108, 128].  h[src] is fetched with ap_gather (free-axis
SBUF gather) from a transposed, group-replicated h-table, then PE-transposed
back to edge-on-partition layout.  Scatter-mean is a PE matmul with a
dst-one-hot stationary accumulated in PSUM per 128-node block, expanded over
(o,i) and reduced on DVE.  Root/bias terms ride as per-block self-tiles.
Between layers a [8, 2560] fp32 AllGather shares h.
"""
import os
import numpy as np
import ml_dtypes

import concourse.bass as bass
from concourse import bacc
import concourse.mybir as mybir
import concourse.tile as tile
from concourse.bass_utils import run_bass_kernel_spmd
from concourse.masks import make_identity

BF16 = ml_dtypes.bfloat16

N, E, B = 20480, 327680, 16
ET, EF = 36, 2
NCORE = 8
NPC = N // NCORE        # 2560 nodes per core
NBLK = NPC // 128       # 20 node blocks per core
GPC = B // NCORE        # 2 graphs per core
NPG = N // B            # 1280 nodes per graph
LAYERS = [(16, 8), (8, 8), (8, 8), (8, 64)]
K108 = 3 * ET           # stacked one-hot rows


def _oi_perm(ci, co):
    """column permutation taking (i,o)-flat [d] -> (o,i)-flat [d]."""
    k = np.arange(ci * co)
    o, i = k // ci, k % ci
    return i * co + o  # new[k=(o,i)] = old[i*co+o]


def _prep(inputs):
    """All host-side numpy preprocessing. Returns (TB, shared, per_core)."""
    x = np.asarray(inputs["x"], np.float32)
    ei = np.asarray(inputs["edge_index"], np.int64)
    etype = np.asarray(inputs["edge_type"], np.int64)
    ea = np.asarray(inputs["edge_attr"], np.float32)
    ct = np.asarray(inputs["cell_type"], np.int64)
    src, dst = ei[0], ei[1]

    deg = np.bincount(dst, minlength=N).astype(np.float32)
    inv_deg = 1.0 / np.maximum(deg, 1.0)

    shared = {"xT": np.ascontiguousarray(x.T).astype(np.float32)}
    for l, (ci, co) in enumerate(LAYERS, 1):
        d = ci * co
        emb = np.asarray(inputs[f"emb{l}"], np.float32)
        wh = np.asarray(inputs[f"wh{l}"], np.float32)
        bh = np.asarray(inputs[f"bh{l}"], np.float32)
        wg = np.asarray(inputs[f"wg{l}"], np.float32)
        bg = np.asarray(inputs[f"bg{l}"], np.float32)
        root = np.asarray(inputs[f"root{l}"], np.float32)
        bias = np.asarray(inputs[f"bias{l}"], np.float32)
        G0 = emb * wh[0][None, :] + np.broadcast_to(wg[0], (ET, d))
        G1 = emb * wh[1][None, :] + np.broadcast_to(wg[1], (ET, d))
        G2 = emb * bh[None, :] + np.broadcast_to(bg, (ET, d))
        p = _oi_perm(ci, co)
        GT = np.concatenate([G0[:, p], G1[:, p], G2[:, p]], axis=0)  # [108, d] (o,i)
        shared[f"GT{l}"] = GT.astype(BF16)
        rr = root.reshape(ci, co).T.reshape(-1)  # (o,i) flat: rr[o*ci+i] = root[i,o]
        shared[f"rootrep{l}"] = np.broadcast_to(rr, (128, d)).astype(BF16).copy()
        shared[f"biasrep{l}"] = np.broadcast_to(bias, (128, co)).astype(np.float32).copy()

    clf_w = np.asarray(inputs["clf_w"], np.float32)   # [64, 1]
    clf_b = np.asarray(inputs["clf_b"], np.float32)   # [1]
    shared["clfw"] = np.broadcast_to(clf_w[:, 0], (2, 64)).astype(np.float32).copy()
    shared["clfb"] = np.full((2, 1), clf_b[0], np.float32)

    # ---- per-core edge organization -------------------------------------
    # window w of an edge = (dst_local_in_block)//64; per (core, block, window)
    # the edges form ceil(n/128) tiles; TB/2 tiles allocated per window.
    per_core_ew = []  # [(core)][block][window] -> array of edge ids
    halfmax = 1
    for c in range(NCORE):
        lo = c * NPC
        em = np.where((dst >= lo) & (dst < lo + NPC))[0]
        dl = dst[em] - lo
        order = np.argsort(dl, kind="stable")
        em, dl = em[order], dl[order]
        blocks = []
        for b in range(NBLK):
            sel = (dl // 128) == b
            ebm, dbm = em[sel], dl[sel]
            wsel = ((dbm % 128) // 64) == 0
            blocks.append((ebm[wsel], ebm[~wsel]))
            halfmax = max(halfmax, -(-len(ebm[wsel]) // 128), -(-len(ebm[~wsel]) // 128))
        per_core_ew.append(blocks)
    TB = 2 * halfmax        # tiles per block (even), window w gets tiles [w*TB/2,(w+1)*TB/2)
    T = NBLK * TB           # message tiles per core per layer
    assert T % 8 == 0

    per_core = []
    for c in range(NCORE):
        lo = c * NPC
        oh = np.zeros((K108, T * 128), BF16)
        d1 = np.zeros((128, T * 64), BF16)
        dgd = np.zeros((128, NBLK * 128), BF16)
        gidx = np.zeros((128, T), np.int16)
        for b in range(NBLK):
            for w in (0, 1):
                edges = per_core_ew[c][b][w]
                for t in range(TB // 2):
                    tau = b * TB + w * (TB // 2) + t
                    seg = edges[t * 128:(t + 1) * 128]
                    n = len(seg)
                    if n:
                        p = np.arange(n)
                        tt = etype[seg]
                        cols = tau * 128 + p
                        oh[tt, cols] = ea[seg, 0].astype(BF16)
                        oh[ET + tt, cols] = ea[seg, 1].astype(BF16)
                        oh[2 * ET + tt, cols] = BF16(1.0)
                        d1[p, tau * 64 + (dst[seg] - lo - b * 128 - w * 64)] = BF16(1.0)
                    # gather indices (wrapped per 16 partitions within group g=tau%8)
                    g = tau % 8
                    q = tau // 8
                    j = np.arange(128)
                    srcs = np.zeros(128, np.int16)
                    srcs[:n] = src[seg].astype(np.int16)
                    gidx[16 * g + (j % 16), q * 8 + j // 16] = srcs
            pb = np.arange(128)
            dgd[pb, b * 128 + pb] = deg[lo + b * 128 + pb].astype(BF16)

        xl = np.zeros((128, NBLK * 16), BF16)
        ivd = np.zeros((128, NBLK), np.float32)
        g2 = np.zeros((128, NBLK * 2), BF16)
        for b in range(NBLK):
            nodes = lo + b * 128 + np.arange(128)
            xl[:, b * 16:(b + 1) * 16] = x[nodes].astype(BF16)
            ivd[:, b] = inv_deg[nodes]
            g2[:, b * 2 + (b >= 10)] = (ct[nodes] == 1).astype(np.float32).astype(BF16)
        cnt = np.array([[(ct[lo:lo + NPG] == 1).sum()], [(ct[lo + NPG:lo + NPC] == 1).sum()]], np.float32)
        ivc = 1.0 / np.maximum(cnt, 1.0)
        per_core.append({"onehotS": oh, "dst1h": d1, "dstdiag": dgd, "gidx": gidx,
                         "xloc": xl, "invdeg": ivd, "gate2": g2, "invcnt": ivc})
    return TB, shared, per_core


_CACHE = {}


def _build(TB, debug_h=False):
    rep = int(os.environ.get("ATHENA_REPEAT", "1"))
    nocc = bool(os.environ.get("ATHENA_NOCC"))
    nogather = bool(os.environ.get("ATHENA_NOGATHER"))
    key = (TB, debug_h, rep, nocc, nogather)
    if key in _CACHE:
        return _CACHE[key]
    T = NBLK * TB
    NBATCH = T // 8
    dt = mybir.dt
    nc = bacc.Bacc("TRN2", target_bir_lowering=False, num_devices=NCORE)

    xT_d = nc.dram_tensor("xT", [16, N], dt.float32, kind="ExternalInput")
    xl_d = nc.dram_tensor("xloc", [128, NBLK * 16], dt.bfloat16, kind="ExternalInput")
    oh_d = nc.dram_tensor("onehotS", [K108, T * 128], dt.bfloat16, kind="ExternalInput")
    d1_d = nc.dram_tensor("dst1h", [128, T * 64], dt.bfloat16, kind="ExternalInput")
    dg_d = nc.dram_tensor("dstdiag", [128, NBLK * 128], dt.bfloat16, kind="ExternalInput")
    gi_d = nc.dram_tensor("gidx", [128, T], dt.int16, kind="ExternalInput")
    ivd_d = nc.dram_tensor("invdeg", [128, NBLK], dt.float32, kind="ExternalInput")
    g2_d = nc.dram_tensor("gate2", [128, NBLK * 2], dt.bfloat16, kind="ExternalInput")
    ivc_d = nc.dram_tensor("invcnt", [2, 1], dt.float32, kind="ExternalInput")
    cw_d = nc.dram_tensor("clfw", [2, 64], dt.float32, kind="ExternalInput")
    cb_d = nc.dram_tensor("clfb", [2, 1], dt.float32, kind="ExternalInput")
    GT_d, rr_d, br_d = {}, {}, {}
    for l, (ci, co) in enumerate(LAYERS, 1):
        d = ci * co
        GT_d[l] = nc.dram_tensor(f"GT{l}", [K108, d], dt.bfloat16, kind="ExternalInput")
        rr_d[l] = nc.dram_tensor(f"rootrep{l}", [128, d], dt.bfloat16, kind="ExternalInput")
        br_d[l] = nc.dram_tensor(f"biasrep{l}", [128, co], dt.float32, kind="ExternalInput")
    out_d = nc.dram_tensor("out", [2, 1], dt.float32, kind="ExternalOutput")
    hdbg_d = nc.dram_tensor("hdbg", [128, 4 * NBLK * 64], dt.float32,
                            kind="ExternalOutput") if debug_h else None
    hsh_d = nc.dram_tensor("hshard", [8, NPC], dt.float32, kind="Internal")
    hfull_d = nc.dram_tensor("hfull", [NCORE * 8, NPC], dt.float32, kind="Internal",
                             addr_space="Shared")

    with tile.TileContext(nc) as tc:
        with tc.tile_pool(name="const", bufs=1) as cpool, \
             tc.tile_pool(name="stream", bufs=3) as spool, \
             tc.tile_pool(name="work", bufs=3) as wpool, \
             tc.tile_pool(name="ps_s", bufs=2, space="PSUM") as ps_s, \
             tc.tile_pool(name="ps_agg", bufs=2, space="PSUM") as ps_agg, \
             tc.tile_pool(name="ps_tr", bufs=2, space="PSUM") as ps_tr:

            d1 = cpool.tile([128, T * 64], dt.bfloat16)
            nc.sync.dma_start(out=d1[:], in_=d1_d[:])
            dg = cpool.tile([128, NBLK * 128], dt.bfloat16)
            nc.sync.dma_start(out=dg[:], in_=dg_d[:])
            gi = cpool.tile([128, T], dt.int16)
            nc.sync.dma_start(out=gi[:], in_=gi_d[:])
            ivd = cpool.tile([128, NBLK], dt.float32)
            nc.sync.dma_start(out=ivd[:], in_=ivd_d[:])
            hloc = cpool.tile([128, NBLK, 16], dt.bfloat16)
            nc.sync.dma_start(out=hloc[:], in_=xl_d[:].rearrange("p (b i) -> p b i", i=16))
            h4 = cpool.tile([128, NBLK, 64], dt.bfloat16)
            table = cpool.tile([128, N, 1], dt.float32)
            hTsb = cpool.tile([16, NPC], dt.float32)
            ident = cpool.tile([128, 128], dt.float32)
            make_identity(nc, ident[:])
            GT, rr, br = {}, {}, {}
            for l, (ci, co) in enumerate(LAYERS, 1):
                d = ci * co
                GT[l] = cpool.tile([K108, d], dt.bfloat16, tag=f"GT{l}", name=f"GT{l}t")
                nc.sync.dma_start(out=GT[l][:], in_=GT_d[l][:])
                rr[l] = cpool.tile([128, d], dt.bfloat16, tag=f"rr{l}", name=f"rr{l}t")
                nc.sync.dma_start(out=rr[l][:], in_=rr_d[l][:])
                br[l] = cpool.tile([128, co], dt.float32, tag=f"br{l}", name=f"br{l}t")
                nc.sync.dma_start(out=br[l][:], in_=br_d[l][:])
            g2t = cpool.tile([128, NBLK * 2], dt.bfloat16)
            nc.sync.dma_start(out=g2t[:], in_=g2_d[:])

            # layer-1 gather table: xT replicated into all 8 groups
            for g in range(8):
                nc.sync.dma_start(out=table[16 * g:16 * g + 16, :, 0], in_=xT_d[:])

            for _rep in range(rep):
              for l, (ci, co) in enumerate(LAYERS, 1):
                  d = ci * co
                  hdst = h4 if l == 4 else hloc
                  for b in range(NBLK):
                      Pagg = ps_agg.tile([128, d], dt.float32, tag="agg")
                      started = [False, False]
                      for s in range(TB // 2):
                          Ps = ps_s.tile([128, 2 * d], dt.float32, tag="s")
                          taus = (b * TB + 2 * s, b * TB + 2 * s + 1)
                          for u, tau in enumerate(taus):
                              q, g8 = tau // 8, tau % 8
                              if g8 == 0:
                                  ohc = spool.tile([K108, 1024], dt.bfloat16, tag="oh")
                                  nc.sync.dma_start(out=ohc[:], in_=oh_d[:, q * 1024:(q + 1) * 1024])
                                  htg = wpool.tile([128, 128, 1], dt.float32, tag="htg")
                                  if nogather:
                                      nc.vector.memset(htg[:], 0.25)
                                  else:
                                      nc.gpsimd.ap_gather(out_ap=htg[:], in_ap=table[:],
                                                          idxs_ap=gi[:, q * 8:(q + 1) * 8],
                                                          channels=128, num_elems=N, d=1,
                                                          num_idxs=128)
                                  Ptr = ps_tr.tile([128, 128], dt.float32, tag="tr")
                                  nc.tensor.transpose(out=Ptr[:], in_=htg[:, :, 0], identity=ident[:])
                                  htr = wpool.tile([128, 128], dt.bfloat16, tag="htr")
                                  nc.scalar.copy(out=htr[:], in_=Ptr[:])
                                  cur_htr = htr
                                  cur_oh = ohc
                              nc.tensor.matmul(out=Ps[:, u * d:(u + 1) * d],
                                               lhsT=cur_oh[:, g8 * 128:(g8 + 1) * 128],
                                               rhs=GT[l][:], start=True, stop=True)
                          Wsl = wpool.tile([128, 2 * d], dt.bfloat16, tag="W")
                          nc.scalar.activation(out=Wsl[:], in_=Ps[:],
                                               func=mybir.ActivationFunctionType.Relu)
                          V = wpool.tile([128, 2 * d], dt.bfloat16, tag="V")
                          g80 = taus[0] % 8
                          h_in1 = bass.AP(cur_htr.tensor, cur_htr[:].offset + g80 * 16,
                                          [cur_htr[:].ap[0], [16, 2], [0, co], [1, ci]])
                          nc.vector.tensor_tensor(
                              out=V[:].rearrange("p (t o i) -> p t o i", t=2, i=ci),
                              in0=Wsl[:].rearrange("p (t o i) -> p t o i", t=2, i=ci),
                              in1=h_in1, op=mybir.AluOpType.mult)
                          for u, tau in enumerate(taus):
                              w = 0 if (tau - b * TB) < TB // 2 else 1
                              nc.tensor.matmul(out=Pagg[w * 64:(w + 1) * 64, :],
                                               lhsT=d1[:, tau * 64:(tau + 1) * 64],
                                               rhs=V[:, u * d:(u + 1) * d],
                                               start=not started[w], stop=False)
                              started[w] = True
                      # self tile: V_self = root_rep * h_local (bcast over o)
                      Vs = wpool.tile([128, d], dt.bfloat16, tag="V")
                      hb = hloc[:, b, 0:ci]
                      h_self = bass.AP(hb.tensor, hb.offset, [hb.ap[0], [0, co], [1, ci]])
                      nc.vector.tensor_tensor(
                          out=Vs[:].rearrange("p (o i) -> p o i", i=ci),
                          in0=rr[l][:].rearrange("p (o i) -> p o i", i=ci),
                          in1=h_self, op=mybir.AluOpType.mult)
                      nc.tensor.matmul(out=Pagg[:, :], lhsT=dg[:, b * 128:(b + 1) * 128],
                                       rhs=Vs[:], start=False, stop=True)
                      # node update
                      S = wpool.tile([128, co], dt.float32, tag="S")
                      nc.vector.tensor_reduce(out=S[:],
                                              in_=Pagg[:].rearrange("p (o i) -> p o i", i=ci),
                                              axis=mybir.AxisListType.X, op=mybir.AluOpType.add)
                      S2 = wpool.tile([128, co], dt.float32, tag="S2")
                      nc.vector.tensor_scalar(out=S2[:], in0=S[:], scalar1=ivd[:, b:b + 1],
                                              scalar2=None, op0=mybir.AluOpType.mult)
                      S3 = wpool.tile([128, co], dt.float32, tag="S3")
                      nc.vector.tensor_tensor(out=S3[:], in0=S2[:], in1=br[l][:],
                                              op=mybir.AluOpType.add)
                      S4 = wpool.tile([128, co], dt.float32, tag="S4")
                      nc.vector.tensor_scalar(out=S4[:], in0=S3[:], scalar1=0.0, scalar2=None,
                                              op0=mybir.AluOpType.max)
                      nc.vector.tensor_copy(out=hdst[:, b, 0:co], in_=S4[:])
                      if debug_h:
                          nc.sync.dma_start(
                              out=hdbg_d[:][:, ((l - 1) * NBLK + b) * 64:((l - 1) * NBLK + b) * 64 + co],
                              in_=S4[:])
                      if l < 4:
                          Ptr2 = ps_tr.tile([128, 128], dt.float32, tag="tr")
                          nc.tensor.transpose(out=Ptr2[0:co, 0:128], in_=S4[:], identity=ident[:])
                          nc.scalar.copy(out=hTsb[0:co, b * 128:(b + 1) * 128], in_=Ptr2[0:co, 0:128])
                  if l < 4 and nocc:
                      nc.sync.dma_start(out=hsh_d[:], in_=hTsb[0:8, :])
                  if l < 4 and not nocc:
                      nc.sync.dma_start(out=hsh_d[:], in_=hTsb[0:8, :])
                      nc.gpsimd.collective_compute(
                          kind="AllGather", op=mybir.AluOpType.bypass,
                          replica_groups=[list(range(NCORE))],
                          ins=[hsh_d[:]], outs=[hfull_d[:]])
                      hf = hfull_d[:]
                      src_ap = bass.AP(hf.tensor, 0, [[NPC, 8], [8 * NPC, 8], [1, NPC]])
                      for g in range(8):
                          nc.sync.dma_start(
                              out=table[16 * g:16 * g + 8, :, 0].rearrange("p (c n) -> p c n", c=8),
                              in_=src_ap)

            # pooling + classifier
            Pp = ps_tr.tile([128, 128], dt.float32, tag="tr")
            for b in range(NBLK):
                nc.tensor.matmul(out=Pp[0:2, 0:64], lhsT=g2t[:, b * 2:(b + 1) * 2],
                                 rhs=h4[:, b, :], start=(b == 0), stop=(b == NBLK - 1))
            pool = wpool.tile([2, 64], dt.float32, tag="pool")
            ivc = cpool.tile([2, 1], dt.float32)
            nc.sync.dma_start(out=ivc[:], in_=ivc_d[:])
            cw = cpool.tile([2, 64], dt.float32)
            nc.sync.dma_start(out=cw[:], in_=cw_d[:])
            cb = cpool.tile([2, 1], dt.float32)
            nc.sync.dma_start(out=cb[:], in_=cb_d[:])
            nc.vector.tensor_scalar(out=pool[:], in0=Pp[0:2, 0:64], scalar1=ivc[:],
                                    scalar2=None, op0=mybir.AluOpType.mult)
            pz = wpool.tile([2, 64], dt.float32, tag="pz")
            nc.vector.tensor_tensor(out=pz[:], in0=pool[:], in1=cw[:], op=mybir.AluOpType.mult)
            z = wpool.tile([2, 1], dt.float32, tag="z")
            nc.vector.tensor_reduce(out=z[:], in_=pz[:], axis=mybir.AxisListType.X,
                                    op=mybir.AluOpType.add)
            z2 = wpool.tile([2, 1], dt.float32, tag="z2")
            nc.vector.tensor_tensor(out=z2[:], in0=z[:], in1=cb[:], op=mybir.AluOpType.add)
            z3 = wpool.tile([2, 1], dt.float32, tag="z3")
            nc.scalar.activation(out=z3[:], in_=z2[:],
                                 func=mybir.ActivationFunctionType.Sigmoid)
            nc.sync.dma_start(out=out_d[:], in_=z3[:])

    nc.compile()
    _CACHE[key] = nc
    return nc


def kernel(**inputs):
    debug_h = bool(os.environ.get("ATHENA_DEBUG_H"))
    TB, shared, per_core = _prep(inputs)
    nc = _build(TB, debug_h)
    in_maps = []
    for c in range(NCORE):
        m = dict(shared)
        m.update(per_core[c])
        in_maps.append(m)
    res = run_bass_kernel_spmd(nc, in_maps, core_ids=list(range(NCORE)),
                               trace=bool(os.environ.get("ATHENA_TRACE")))
    kernel.last_results = res
    outs = [res.results[c]["out"] for c in range(NCORE)]
    return np.concatenate(outs, axis=0).astype(np.float32)



# revision 7
# speedup vs baseline: 1.2728x; 1.2728x over previous
"""CellSpatialNet (4-layer NNConv GNN) on 8 trn2 NeuronCores.

Strategy: shard nodes+edges by dst across 8 cores (2560 nodes = 2 graphs/core).
Host folds the EdgeNN into per-type tables G0/G1(/G2) [36, d] so that
  W[e] = relu(ef0*G0[t_e] + ef1*G1[t_e] (+ G2[t_e]))
becomes ONE PE matmul per 128-edge tile with a host-built "scaled one-hot"
stationary operand [K, 128] (K=72 when the bias tables are zero, 108 else).
h[src] is fetched with ap_gather (free-axis SBUF gather) from a transposed,
group-replicated h-table, then PE-transposed back to edge-on-partition
layout.  relu+mult are FUSED into one DVE scalar_tensor_tensor per PSUM
slot: V = max(A,0)*h_bcast.  Scatter-mean is a PE matmul with a dst-one-hot
stationary accumulated in PSUM per 128-node block, expanded over (o,i) and
reduced on DVE.  Root/bias terms ride as per-layer precomputed self-tiles.
Between layers a [8, 2560] fp32 AllGather shares h; a dummy AllGather at
kernel start warms the collective stream concurrently with layer-1 compute.
"""
import os
import numpy as np
import ml_dtypes

import concourse.bass as bass
from concourse import bacc
import concourse.mybir as mybir
import concourse.tile as tile
from concourse.bass_utils import run_bass_kernel_spmd
from concourse.masks import make_identity

BF16 = ml_dtypes.bfloat16

N, E, B = 20480, 327680, 16
ET, EF = 36, 2
NCORE = 8
NPC = N // NCORE        # 2560 nodes per core
NBLK = NPC // 128       # 20 node blocks per core
GPC = B // NCORE        # 2 graphs per core
NPG = N // B            # 1280 nodes per graph
LAYERS = [(16, 8), (8, 8), (8, 8), (8, 64)]


def _oi_perm(ci, co):
    """column permutation taking (i,o)-flat [d] -> (o,i)-flat [d]."""
    k = np.arange(ci * co)
    o, i = k // ci, k % ci
    return i * co + o  # new[k=(o,i)] = old[i*co+o]


def _prep(inputs):
    """All host-side numpy preprocessing. Returns (TB, K, shared, per_core)."""
    x = np.asarray(inputs["x"], np.float32)
    ei = np.asarray(inputs["edge_index"], np.int64)
    etype = np.asarray(inputs["edge_type"], np.int64)
    ea = np.asarray(inputs["edge_attr"], np.float32)
    ct = np.asarray(inputs["cell_type"], np.int64)
    src, dst = ei[0], ei[1]

    deg = np.bincount(dst, minlength=N).astype(np.float32)
    inv_deg = 1.0 / np.maximum(deg, 1.0)

    shared = {"xT": np.ascontiguousarray(x.T).astype(np.float32)}
    g2zero = True
    Gts = {}
    for l, (ci, co) in enumerate(LAYERS, 1):
        d = ci * co
        emb = np.asarray(inputs[f"emb{l}"], np.float32)
        wh = np.asarray(inputs[f"wh{l}"], np.float32)
        bh = np.asarray(inputs[f"bh{l}"], np.float32)
        wg = np.asarray(inputs[f"wg{l}"], np.float32)
        bg = np.asarray(inputs[f"bg{l}"], np.float32)
        root = np.asarray(inputs[f"root{l}"], np.float32)
        bias = np.asarray(inputs[f"bias{l}"], np.float32)
        G0 = emb * wh[0][None, :] + np.broadcast_to(wg[0], (ET, d))
        G1 = emb * wh[1][None, :] + np.broadcast_to(wg[1], (ET, d))
        G2 = emb * bh[None, :] + np.broadcast_to(bg, (ET, d))
        if np.abs(G2).max() > 0:
            g2zero = False
        p = _oi_perm(ci, co)
        Gts[l] = (G0[:, p], G1[:, p], G2[:, p])
        rr = root.reshape(ci, co).T.reshape(-1)  # (o,i) flat: rr[o*ci+i] = root[i,o]
        shared[f"rootrep{l}"] = np.broadcast_to(rr, (128, d)).astype(BF16).copy()
        shared[f"biasrep{l}"] = np.broadcast_to(bias, (128, co)).astype(np.float32).copy()
    K = 2 * ET if g2zero else 3 * ET
    for l, (ci, co) in enumerate(LAYERS, 1):
        G0p, G1p, G2p = Gts[l]
        parts = [G0p, G1p] if g2zero else [G0p, G1p, G2p]
        shared[f"GT{l}"] = np.concatenate(parts, axis=0).astype(BF16)  # [K, d]

    clf_w = np.asarray(inputs["clf_w"], np.float32)   # [64, 1]
    clf_b = np.asarray(inputs["clf_b"], np.float32)   # [1]
    shared["clfw"] = np.broadcast_to(clf_w[:, 0], (2, 64)).astype(np.float32).copy()
    shared["clfb"] = np.full((2, 1), clf_b[0], np.float32)

    # ---- per-core edge organization -------------------------------------
    # window w of an edge = (dst_local_in_block)//64; per (core, block, window)
    # the edges form ceil(n/128) tiles; TB/2 tiles allocated per window.
    per_core_ew = []  # [(core)][block][window] -> array of edge ids
    halfmax = 1
    for c in range(NCORE):
        lo = c * NPC
        em = np.where((dst >= lo) & (dst < lo + NPC))[0]
        dl = dst[em] - lo
        order = np.argsort(dl, kind="stable")
        em, dl = em[order], dl[order]
        blocks = []
        for b in range(NBLK):
            sel = (dl // 128) == b
            ebm, dbm = em[sel], dl[sel]
            wsel = ((dbm % 128) // 64) == 0
            blocks.append((ebm[wsel], ebm[~wsel]))
            halfmax = max(halfmax, -(-len(ebm[wsel]) // 128), -(-len(ebm[~wsel]) // 128))
        per_core_ew.append(blocks)
    TB = 2 * halfmax        # tiles per block (even), window w gets tiles [w*TB/2,(w+1)*TB/2)
    T = NBLK * TB           # message tiles per core per layer
    assert T % 8 == 0

    per_core = []
    for c in range(NCORE):
        lo = c * NPC
        oh = np.zeros((K, T * 128), BF16)
        d1 = np.zeros((128, T * 64), BF16)
        dgd = np.zeros((128, NBLK * 128), BF16)
        gidx = np.zeros((128, T), np.int16)
        for b in range(NBLK):
            for w in (0, 1):
                edges = per_core_ew[c][b][w]
                for t in range(TB // 2):
                    tau = b * TB + w * (TB // 2) + t
                    seg = edges[t * 128:(t + 1) * 128]
                    n = len(seg)
                    if n:
                        p = np.arange(n)
                        tt = etype[seg]
                        cols = tau * 128 + p
                        oh[tt, cols] = ea[seg, 0].astype(BF16)
                        oh[ET + tt, cols] = ea[seg, 1].astype(BF16)
                        if K == 3 * ET:
                            oh[2 * ET + tt, cols] = BF16(1.0)
                        d1[p, tau * 64 + (dst[seg] - lo - b * 128 - w * 64)] = BF16(1.0)
                    # gather indices (wrapped per 16 partitions within group g=tau%8)
                    g = tau % 8
                    q = tau // 8
                    j = np.arange(128)
                    srcs = np.zeros(128, np.int16)
                    srcs[:n] = src[seg].astype(np.int16)
                    gidx[16 * g + (j % 16), q * 8 + j // 16] = srcs
            pb = np.arange(128)
            dgd[pb, b * 128 + pb] = deg[lo + b * 128 + pb].astype(BF16)

        xl = np.zeros((128, NBLK * 16), BF16)
        ivd = np.zeros((128, NBLK), np.float32)
        g2 = np.zeros((128, NBLK * 2), BF16)
        for b in range(NBLK):
            nodes = lo + b * 128 + np.arange(128)
            xl[:, b * 16:(b + 1) * 16] = x[nodes].astype(BF16)
            ivd[:, b] = inv_deg[nodes]
            g2[:, b * 2 + (b >= 10)] = (ct[nodes] == 1).astype(np.float32).astype(BF16)
        cnt = np.array([[(ct[lo:lo + NPG] == 1).sum()], [(ct[lo + NPG:lo + NPC] == 1).sum()]], np.float32)
        ivc = 1.0 / np.maximum(cnt, 1.0)
        per_core.append({"onehotS": oh, "dst1h": d1, "dstdiag": dgd, "gidx": gidx,
                         "xloc": xl, "invdeg": ivd, "gate2": g2, "invcnt": ivc})
    return TB, K, shared, per_core


_CACHE = {}


def _build(TB, K, debug_h=False):
    nocc = bool(os.environ.get("ATHENA_NOCC"))
    nogather = bool(os.environ.get("ATHENA_NOGATHER"))
    nowarm = bool(os.environ.get("ATHENA_NOWARM"))
    key = (TB, K, debug_h, nocc, nogather, nowarm)
    if key in _CACHE:
        return _CACHE[key]
    T = NBLK * TB
    NG = T // 8                       # 8-tile groups per layer
    dt = mybir.dt
    nc = bacc.Bacc("TRN2", target_bir_lowering=False, num_devices=NCORE)

    xT_d = nc.dram_tensor("xT", [16, N], dt.float32, kind="ExternalInput")
    xl_d = nc.dram_tensor("xloc", [128, NBLK * 16], dt.bfloat16, kind="ExternalInput")
    oh_d = nc.dram_tensor("onehotS", [K, T * 128], dt.bfloat16, kind="ExternalInput")
    d1_d = nc.dram_tensor("dst1h", [128, T * 64], dt.bfloat16, kind="ExternalInput")
    dg_d = nc.dram_tensor("dstdiag", [128, NBLK * 128], dt.bfloat16, kind="ExternalInput")
    gi_d = nc.dram_tensor("gidx", [128, T], dt.int16, kind="ExternalInput")
    ivd_d = nc.dram_tensor("invdeg", [128, NBLK], dt.float32, kind="ExternalInput")
    g2_d = nc.dram_tensor("gate2", [128, NBLK * 2], dt.bfloat16, kind="ExternalInput")
    ivc_d = nc.dram_tensor("invcnt", [2, 1], dt.float32, kind="ExternalInput")
    cw_d = nc.dram_tensor("clfw", [2, 64], dt.float32, kind="ExternalInput")
    cb_d = nc.dram_tensor("clfb", [2, 1], dt.float32, kind="ExternalInput")
    GT_d, rr_d, br_d = {}, {}, {}
    for l, (ci, co) in enumerate(LAYERS, 1):
        d = ci * co
        GT_d[l] = nc.dram_tensor(f"GT{l}", [K, d], dt.bfloat16, kind="ExternalInput")
        rr_d[l] = nc.dram_tensor(f"rootrep{l}", [128, d], dt.bfloat16, kind="ExternalInput")
        br_d[l] = nc.dram_tensor(f"biasrep{l}", [128, co], dt.float32, kind="ExternalInput")
    out_d = nc.dram_tensor("out", [2, 1], dt.float32, kind="ExternalOutput")
    hdbg_d = nc.dram_tensor("hdbg", [128, 4 * NBLK * 64], dt.float32,
                            kind="ExternalOutput") if debug_h else None
    hsh_d = nc.dram_tensor("hshard", [8, NPC], dt.float32, kind="Internal")
    hfull_d = nc.dram_tensor("hfull", [NCORE * 8, NPC], dt.float32, kind="Internal",
                             addr_space="Shared")
    warm_in_d = nc.dram_tensor("warmin", [8, 16], dt.float32, kind="Internal")
    warm_out_d = nc.dram_tensor("warmout", [NCORE * 8, 16], dt.float32, kind="Internal",
                                addr_space="Shared")

    with tile.TileContext(nc) as tc:
        with tc.tile_pool(name="const", bufs=1) as cpool, \
             tc.tile_pool(name="stream", bufs=3) as spool, \
             tc.tile_pool(name="work", bufs=4) as wpool, \
             tc.tile_pool(name="ps_s", bufs=4, space="PSUM") as ps_s, \
             tc.tile_pool(name="ps_agg", bufs=2, space="PSUM") as ps_agg, \
             tc.tile_pool(name="ps_tr", bufs=2, space="PSUM") as ps_tr:

            # ---- collective-stream warmup (overlaps layer-1 compute) ----
            if not nocc and not nowarm:
                wt = cpool.tile([8, 16], dt.float32)
                nc.vector.memset(wt[:], 0.0)
                nc.sync.dma_start(out=warm_in_d[:], in_=wt[:])
                nc.gpsimd.collective_compute(
                    kind="AllGather", op=mybir.AluOpType.bypass,
                    replica_groups=[list(range(NCORE))],
                    ins=[warm_in_d[:]], outs=[warm_out_d[:]])

            # ---- critical-path preamble ---------------------------------
            GT, rr, br = {}, {}, {}
            for l, (ci, co) in enumerate(LAYERS, 1):
                d = ci * co
                GT[l] = cpool.tile([K, d], dt.bfloat16, tag=f"GT{l}", name=f"GT{l}t")
                nc.sync.dma_start(out=GT[l][:], in_=GT_d[l][:])
            gi = cpool.tile([128, T], dt.int16)
            nc.sync.dma_start(out=gi[:], in_=gi_d[:])
            # layer-1 gather table: one HBM read of xT, then on-chip replication
            table = cpool.tile([128, N, 1], dt.float32)
            nc.sync.dma_start(out=table[0:16, :, 0], in_=xT_d[:])
            for g in range(1, 8):
                nc.sync.dma_start(out=table[16 * g:16 * g + 16, :, 0],
                                  in_=table[0:16, :, 0])
            hloc = cpool.tile([128, NBLK, 16], dt.bfloat16)
            nc.sync.dma_start(out=hloc[:], in_=xl_d[:].rearrange("p (b i) -> p b i", i=16))
            d1 = cpool.tile([128, T * 64], dt.bfloat16)
            NCH = 5
            for c in range(NCH):
                c0, c1 = (T // NCH) * c * 64, (T // NCH) * (c + 1) * 64
                nc.sync.dma_start(out=d1[:, c0:c1], in_=d1_d[:, c0:c1])
            dg = cpool.tile([128, NBLK * 128], dt.bfloat16)
            nc.sync.dma_start(out=dg[:], in_=dg_d[:])
            ivd = cpool.tile([128, NBLK], dt.float32)
            nc.sync.dma_start(out=ivd[:], in_=ivd_d[:])
            for l in range(1, 5):
                d = LAYERS[l - 1][0] * LAYERS[l - 1][1]
                rr[l] = cpool.tile([128, d], dt.bfloat16, tag=f"rr{l}", name=f"rr{l}t")
                nc.sync.dma_start(out=rr[l][:], in_=rr_d[l][:])
                br[l] = cpool.tile([128, LAYERS[l - 1][1]], dt.float32, tag=f"br{l}",
                                   name=f"br{l}t")
                nc.sync.dma_start(out=br[l][:], in_=br_d[l][:])
            g2t = cpool.tile([128, NBLK * 2], dt.bfloat16)
            nc.sync.dma_start(out=g2t[:], in_=g2_d[:])
            h4 = cpool.tile([128, NBLK, 64], dt.bfloat16)
            hTsb = cpool.tile([16, NPC], dt.float32)
            ident = cpool.tile([128, 128], dt.float32)
            make_identity(nc, ident[:])
            identb = cpool.tile([128, 128], dt.bfloat16)
            make_identity(nc, identb[:])

            for l, (ci, co) in enumerate(LAYERS, 1):
                d = ci * co
                nts = max(1, 512 // d)        # message tiles per PSUM slot
                hdst = h4 if l == 4 else hloc
                # per-layer self-tiles: Vs_all[:, b, :] = rootrep * h_b (bcast over o)
                vsall = cpool.tile([128, NBLK, 512], dt.bfloat16, tag="vsall",
                                   name=f"vsall{l}")
                hb = hloc[:, 0, 0:ci]
                h_self = bass.AP(hb.tensor, hb.offset,
                                 [hb.ap[0], [16, NBLK], [0, co], [1, ci]])
                rr0 = rr[l][:]
                rr_b = bass.AP(rr0.tensor, rr0.offset,
                               [rr0.ap[0], [0, NBLK], [1, d]])
                nc.vector.tensor_tensor(
                    out=vsall[:, :, 0:d].rearrange("p b (o i) -> p b o i", i=ci),
                    in0=rr_b.rearrange("p b (o i) -> p b o i", i=ci),
                    in1=h_self, op=mybir.AluOpType.mult)

                cur_agg = {}      # block -> (psum tile, [started_w0, started_w1])
                for q in range(NG):
                    ohc = spool.tile([K, 1024], dt.bfloat16, tag="oh")
                    nc.sync.dma_start(out=ohc[:], in_=oh_d[:, q * 1024:(q + 1) * 1024])
                    htg = wpool.tile([128, 128, 1], dt.float32, tag="htg")
                    if nogather:
                        nc.vector.memset(htg[:], 0.25)
                    else:
                        nc.gpsimd.ap_gather(out_ap=htg[:], in_ap=table[:],
                                            idxs_ap=gi[:, q * 8:(q + 1) * 8],
                                            channels=128, num_elems=N, d=1,
                                            num_idxs=128)
                    Ptr = ps_tr.tile([128, 128], dt.float32, tag="tr")
                    nc.tensor.transpose(out=Ptr[:], in_=htg[:, :, 0], identity=ident[:])
                    htr = wpool.tile([128, 128], dt.bfloat16, tag="htr")
                    nc.scalar.copy(out=htr[:], in_=Ptr[:])

                    # A) all W matmuls of the group (keeps PE stream dense)
                    Pss = []
                    for s in range(8 // nts):
                        Ps = ps_s.tile([128, 512], dt.float32, tag="s",
                                       name=f"s{l}_{q}_{s}")
                        for t in range(nts):
                            g8 = s * nts + t
                            nc.tensor.matmul(out=Ps[:, t * d:(t + 1) * d],
                                             lhsT=ohc[:, g8 * 128:(g8 + 1) * 128],
                                             rhs=GT[l][:], start=True, stop=True)
                        Pss.append(Ps)
                    # B) V = relu(A) * h[src]  (relu-evac on ScalarE, mult on DVE)
                    Vss = []
                    for s in range(8 // nts):
                        Wsl = wpool.tile([128, 512], dt.bfloat16, tag="W",
                                         name=f"W{l}_{q}_{s}")
                        nc.scalar.activation(out=Wsl[:], in_=Pss[s][:],
                                             func=mybir.ActivationFunctionType.Relu)
                        V = wpool.tile([128, 512], dt.bfloat16, tag="V",
                                       name=f"V{l}_{q}_{s}")
                        g80 = s * nts
                        h_in1 = bass.AP(htr.tensor, htr[:].offset + g80 * 16,
                                        [htr[:].ap[0], [16, nts], [0, co], [1, ci]])
                        nc.vector.tensor_tensor(
                            out=V[:].rearrange("p (t o i) -> p t o i", t=nts, i=ci),
                            in0=Wsl[:].rearrange("p (t o i) -> p t o i", t=nts, i=ci),
                            in1=h_in1, op=mybir.AluOpType.mult)
                        Vss.append(V)
                    # C) scatter-accumulate per tile
                    for g8 in range(8):
                        s, t = g8 // nts, g8 % nts
                        tau = 8 * q + g8
                        b = tau // TB
                        w = 0 if (tau - b * TB) < TB // 2 else 1
                        if b not in cur_agg:
                            Pagg_new = ps_agg.tile([128, 512], dt.float32,
                                                   tag="agg", name=f"agg{l}_{b}")
                            cur_agg[b] = (Pagg_new, [False, False])
                        Pagg, started = cur_agg[b]
                        nc.tensor.matmul(out=Pagg[w * 64:(w + 1) * 64, 0:d],
                                         lhsT=d1[:, tau * 64:(tau + 1) * 64],
                                         rhs=Vss[s][:, t * d:(t + 1) * d],
                                         start=not started[w], stop=False)
                        started[w] = True
                        if tau == b * TB + TB - 1:
                            # ---- block tail: finish node update -----
                            nc.tensor.matmul(out=Pagg[:, 0:d],
                                             lhsT=dg[:, b * 128:(b + 1) * 128],
                                             rhs=vsall[:, b, 0:d],
                                             start=False, stop=True)
                            S = wpool.tile([128, co], dt.float32, tag="S",
                                           name=f"S{l}_{b}")
                            nc.vector.tensor_reduce(
                                out=S[:],
                                in_=Pagg[:, 0:d].rearrange("p (o i) -> p o i", i=ci),
                                axis=mybir.AxisListType.X, op=mybir.AluOpType.add)
                            S2 = wpool.tile([128, co], dt.float32, tag="S2",
                                            name=f"S2{l}_{b}")
                            nc.scalar.activation(out=S2[:], in_=S[:],
                                                 func=mybir.ActivationFunctionType.Copy,
                                                 scale=ivd[:, b:b + 1])
                            S3 = wpool.tile([128, co], dt.float32, tag="S3",
                                            name=f"S3{l}_{b}")
                            nc.vector.tensor_tensor(out=S3[:], in0=S2[:],
                                                    in1=br[l][:],
                                                    op=mybir.AluOpType.add)
                            S4 = wpool.tile([128, co], dt.float32, tag="S4",
                                            name=f"S4{l}_{b}")
                            nc.vector.tensor_scalar(out=S4[:], in0=S3[:],
                                                    scalar1=0.0, scalar2=None,
                                                    op0=mybir.AluOpType.max)
                            nc.vector.tensor_copy(out=hdst[:, b, 0:co], in_=S4[:])
                            if debug_h:
                                nc.sync.dma_start(
                                    out=hdbg_d[:][:, ((l - 1) * NBLK + b) * 64:
                                                  ((l - 1) * NBLK + b) * 64 + co],
                                    in_=S4[:])
                            del cur_agg[b]

                # ---- share h across cores, refill gather table ----------
                if l < 4:
                    # deferred transposes h[:, b, :co] -> hTsb (off the PE
                    # critical path during the group loop)
                    for b in range(NBLK):
                        Ptr2 = ps_tr.tile([128, 128], dt.bfloat16, tag="tr",
                                          name=f"tr{l}_{b}")
                        nc.tensor.transpose(out=Ptr2[0:co, 0:128],
                                            in_=hdst[:, b, 0:co],
                                            identity=identb[:])
                        nc.scalar.copy(out=hTsb[0:co, b * 128:(b + 1) * 128],
                                       in_=Ptr2[0:co, 0:128])
                    nc.sync.dma_start(out=hsh_d[:], in_=hTsb[0:8, :])
                    if not nocc:
                        nc.gpsimd.collective_compute(
                            kind="AllGather", op=mybir.AluOpType.bypass,
                            replica_groups=[list(range(NCORE))],
                            ins=[hsh_d[:]], outs=[hfull_d[:]])
                        hf = hfull_d[:]
                        src_ap = bass.AP(hf.tensor, 0, [[NPC, 8], [8 * NPC, 8], [1, NPC]])
                        for g in range(8):
                            nc.sync.dma_start(
                                out=table[16 * g:16 * g + 8, :, 0]
                                    .rearrange("p (c n) -> p c n", c=8),
                                in_=src_ap)

            # ---- pooling + classifier -----------------------------------
            Pp = ps_tr.tile([128, 128], dt.float32, tag="tr")
            for b in range(NBLK):
                nc.tensor.matmul(out=Pp[0:2, 0:64], lhsT=g2t[:, b * 2:(b + 1) * 2],
                                 rhs=h4[:, b, :], start=(b == 0), stop=(b == NBLK - 1))
            pool = wpool.tile([2, 64], dt.float32, tag="pool")
            ivc = cpool.tile([2, 1], dt.float32)
            nc.sync.dma_start(out=ivc[:], in_=ivc_d[:])
            cw = cpool.tile([2, 64], dt.float32)
            nc.sync.dma_start(out=cw[:], in_=cw_d[:])
            cb = cpool.tile([2, 1], dt.float32)
            nc.sync.dma_start(out=cb[:], in_=cb_d[:])
            nc.vector.tensor_scalar(out=pool[:], in0=Pp[0:2, 0:64], scalar1=ivc[:],
                                    scalar2=None, op0=mybir.AluOpType.mult)
            pz = wpool.tile([2, 64], dt.float32, tag="pz")
            nc.vector.tensor_tensor(out=pz[:], in0=pool[:], in1=cw[:], op=mybir.AluOpType.mult)
            z = wpool.tile([2, 1], dt.float32, tag="z")
            nc.vector.tensor_reduce(out=z[:], in_=pz[:], axis=mybir.AxisListType.X,
                                    op=mybir.AluOpType.add)
            z2 = wpool.tile([2, 1], dt.float32, tag="z2")
            nc.vector.tensor_tensor(out=z2[:], in0=z[:], in1=cb[:], op=mybir.AluOpType.add)
            z3 = wpool.tile([2, 1], dt.float32, tag="z3")
            nc.scalar.activation(out=z3[:], in_=z2[:],
                                 func=mybir.ActivationFunctionType.Sigmoid)
            nc.sync.dma_start(out=out_d[:], in_=z3[:])

    nc.compile()
    _CACHE[key] = nc
    return nc


def kernel(**inputs):
    debug_h = bool(os.environ.get("ATHENA_DEBUG_H"))
    TB, K, shared, per_core = _prep(inputs)
    nc = _build(TB, K, debug_h)
    in_maps = []
    for c in range(NCORE):
        m = dict(shared)
        m.update(per_core[c])
        in_maps.append(m)
    res = run_bass_kernel_spmd(nc, in_maps, core_ids=list(range(NCORE)),
                               trace=bool(os.environ.get("ATHENA_TRACE")))
    kernel.last_results = res
    outs = [res.results[c]["out"] for c in range(NCORE)]
    return np.concatenate(outs, axis=0).astype(np.float32)


# revision 16
# speedup vs baseline: 1.2940x; 1.0167x over previous
"""CellSpatialNet (4-layer NNConv GNN) on 8 trn2 NeuronCores.

Strategy: shard nodes+edges by dst across 8 cores (2560 nodes = 2 graphs/core).
Host folds the EdgeNN into per-type tables G0/G1(/G2) [36, d] so that
  W[e] = relu(ef0*G0[t_e] + ef1*G1[t_e] (+ G2[t_e]))
becomes ONE PE matmul per 128-edge tile with a host-built "scaled one-hot"
stationary operand [K, 128] (K=72 when the bias tables are zero, 108 else).
h[src] is fetched with ap_gather (free-axis SBUF gather) from a transposed,
group-replicated h-table, then PE-transposed back to edge-on-partition
layout.  relu+mult are FUSED into one DVE scalar_tensor_tensor per PSUM
slot: V = max(A,0)*h_bcast.  Scatter-mean is a PE matmul with a dst-one-hot
stationary accumulated in PSUM per 128-node block, expanded over (o,i) and
reduced on DVE.  Root/bias terms ride as per-layer precomputed self-tiles.
Between layers a [8, 2560] fp32 AllGather shares h; a dummy AllGather at
kernel start warms the collective stream concurrently with layer-1 compute.
"""
import os
import numpy as np
import ml_dtypes

import concourse.bass as bass
from concourse import bacc
import concourse.mybir as mybir
import concourse.tile as tile
from concourse.bass_utils import run_bass_kernel_spmd
from concourse.masks import make_identity

BF16 = ml_dtypes.bfloat16

N, E, B = 20480, 327680, 16
ET, EF = 36, 2
NCORE = 8
NPC = N // NCORE        # 2560 nodes per core
NBLK = NPC // 128       # 20 node blocks per core
GPC = B // NCORE        # 2 graphs per core
NPG = N // B            # 1280 nodes per graph
LAYERS = [(16, 8), (8, 8), (8, 8), (8, 64)]


def _oi_perm(ci, co):
    """column permutation taking (i,o)-flat [d] -> (o,i)-flat [d]."""
    k = np.arange(ci * co)
    o, i = k // ci, k % ci
    return i * co + o  # new[k=(o,i)] = old[i*co+o]


def _prep(inputs):
    """All host-side numpy preprocessing. Returns (TB, K, shared, per_core)."""
    x = np.asarray(inputs["x"], np.float32)
    ei = np.asarray(inputs["edge_index"], np.int64)
    etype = np.asarray(inputs["edge_type"], np.int64)
    ea = np.asarray(inputs["edge_attr"], np.float32)
    ct = np.asarray(inputs["cell_type"], np.int64)
    src, dst = ei[0], ei[1]

    deg = np.bincount(dst, minlength=N).astype(np.float32)
    inv_deg = 1.0 / np.maximum(deg, 1.0)

    shared = {"xT": np.ascontiguousarray(x.T).astype(np.float32)}
    g2zero = True
    Gts = {}
    for l, (ci, co) in enumerate(LAYERS, 1):
        d = ci * co
        emb = np.asarray(inputs[f"emb{l}"], np.float32)
        wh = np.asarray(inputs[f"wh{l}"], np.float32)
        bh = np.asarray(inputs[f"bh{l}"], np.float32)
        wg = np.asarray(inputs[f"wg{l}"], np.float32)
        bg = np.asarray(inputs[f"bg{l}"], np.float32)
        root = np.asarray(inputs[f"root{l}"], np.float32)
        bias = np.asarray(inputs[f"bias{l}"], np.float32)
        G0 = emb * wh[0][None, :] + np.broadcast_to(wg[0], (ET, d))
        G1 = emb * wh[1][None, :] + np.broadcast_to(wg[1], (ET, d))
        G2 = emb * bh[None, :] + np.broadcast_to(bg, (ET, d))
        if np.abs(G2).max() > 0:
            g2zero = False
        p = _oi_perm(ci, co)
        Gts[l] = (G0[:, p], G1[:, p], G2[:, p])
        rr = root.reshape(ci, co).T.reshape(-1)  # (o,i) flat: rr[o*ci+i] = root[i,o]
        shared[f"rootrep{l}"] = np.broadcast_to(rr, (128, d)).astype(BF16).copy()
        shared[f"biasrep{l}"] = np.broadcast_to(bias, (128, co)).astype(np.float32).copy()
    K = 2 * ET if g2zero else 3 * ET
    for l, (ci, co) in enumerate(LAYERS, 1):
        G0p, G1p, G2p = Gts[l]
        parts = [G0p, G1p] if g2zero else [G0p, G1p, G2p]
        shared[f"GT{l}"] = np.concatenate(parts, axis=0).astype(BF16)  # [K, d]

    clf_w = np.asarray(inputs["clf_w"], np.float32)   # [64, 1]
    clf_b = np.asarray(inputs["clf_b"], np.float32)   # [1]
    shared["clfw"] = np.broadcast_to(clf_w[:, 0], (2, 64)).astype(np.float32).copy()
    shared["clfb"] = np.full((2, 1), clf_b[0], np.float32)

    # ---- per-core edge organization -------------------------------------
    # window w of an edge = (dst_local_in_block)//64; per (core, block, window)
    # the edges form ceil(n/128) tiles; TB/2 tiles allocated per window.
    per_core_ew = []  # [(core)][block][window] -> array of edge ids
    halfmax = 1
    for c in range(NCORE):
        lo = c * NPC
        em = np.where((dst >= lo) & (dst < lo + NPC))[0]
        dl = dst[em] - lo
        order = np.argsort(dl, kind="stable")
        em, dl = em[order], dl[order]
        blocks = []
        for b in range(NBLK):
            sel = (dl // 128) == b
            ebm, dbm = em[sel], dl[sel]
            wsel = ((dbm % 128) // 64) == 0
            blocks.append((ebm[wsel], ebm[~wsel]))
            halfmax = max(halfmax, -(-len(ebm[wsel]) // 128), -(-len(ebm[~wsel]) // 128))
        per_core_ew.append(blocks)
    TB = 2 * halfmax        # tiles per block (even), window w gets tiles [w*TB/2,(w+1)*TB/2)
    T = NBLK * TB           # message tiles per core per layer
    assert T % 8 == 0

    per_core = []
    for c in range(NCORE):
        lo = c * NPC
        oh = np.zeros((K, T * 128), BF16)
        d1 = np.zeros((128, T * 64), BF16)
        dgd = np.zeros((128, NBLK * 128), BF16)
        gidx = np.zeros((128, T), np.int16)
        for b in range(NBLK):
            for w in (0, 1):
                edges = per_core_ew[c][b][w]
                for t in range(TB // 2):
                    tau = b * TB + w * (TB // 2) + t
                    seg = edges[t * 128:(t + 1) * 128]
                    n = len(seg)
                    if n:
                        p = np.arange(n)
                        tt = etype[seg]
                        cols = tau * 128 + p
                        oh[tt, cols] = ea[seg, 0].astype(BF16)
                        oh[ET + tt, cols] = ea[seg, 1].astype(BF16)
                        if K == 3 * ET:
                            oh[2 * ET + tt, cols] = BF16(1.0)
                        d1[p, tau * 64 + (dst[seg] - lo - b * 128 - w * 64)] = BF16(1.0)
                    # gather indices (wrapped per 16 partitions within group g=tau%8)
                    g = tau % 8
                    q = tau // 8
                    j = np.arange(128)
                    srcs = np.zeros(128, np.int16)
                    srcs[:n] = src[seg].astype(np.int16)
                    gidx[16 * g + (j % 16), q * 8 + j // 16] = srcs
            pb = np.arange(128)
            dgd[pb, b * 128 + pb] = deg[lo + b * 128 + pb].astype(BF16)

        xl = np.zeros((128, NBLK * 16), BF16)
        ivd = np.zeros((128, NBLK), np.float32)
        g2 = np.zeros((128, NBLK * 2), BF16)
        for b in range(NBLK):
            nodes = lo + b * 128 + np.arange(128)
            xl[:, b * 16:(b + 1) * 16] = x[nodes].astype(BF16)
            ivd[:, b] = inv_deg[nodes]
            g2[:, b * 2 + (b >= 10)] = (ct[nodes] == 1).astype(np.float32).astype(BF16)
        cnt = np.array([[(ct[lo:lo + NPG] == 1).sum()], [(ct[lo + NPG:lo + NPC] == 1).sum()]], np.float32)
        ivc = 1.0 / np.maximum(cnt, 1.0)
        per_core.append({"onehotS": oh, "dst1h": d1, "dstdiag": dgd, "gidx": gidx,
                         "xloc": xl, "invdeg": ivd, "gate2": g2, "invcnt": ivc})
    return TB, K, shared, per_core


_CACHE = {}


def _build(TB, K, debug_h=False):
    nocc = bool(os.environ.get("ATHENA_NOCC"))
    nogather = bool(os.environ.get("ATHENA_NOGATHER"))
    nowarm = bool(os.environ.get("ATHENA_NOWARM"))
    key = (TB, K, debug_h, nocc, nogather, nowarm)
    if key in _CACHE:
        return _CACHE[key]
    T = NBLK * TB
    NG = T // 8                       # 8-tile groups per layer
    dt = mybir.dt
    nc = bacc.Bacc("TRN2", target_bir_lowering=False, num_devices=NCORE)

    xT_d = nc.dram_tensor("xT", [16, N], dt.float32, kind="ExternalInput")
    xl_d = nc.dram_tensor("xloc", [128, NBLK * 16], dt.bfloat16, kind="ExternalInput")
    oh_d = nc.dram_tensor("onehotS", [K, T * 128], dt.bfloat16, kind="ExternalInput")
    d1_d = nc.dram_tensor("dst1h", [128, T * 64], dt.bfloat16, kind="ExternalInput")
    dg_d = nc.dram_tensor("dstdiag", [128, NBLK * 128], dt.bfloat16, kind="ExternalInput")
    gi_d = nc.dram_tensor("gidx", [128, T], dt.int16, kind="ExternalInput")
    ivd_d = nc.dram_tensor("invdeg", [128, NBLK], dt.float32, kind="ExternalInput")
    g2_d = nc.dram_tensor("gate2", [128, NBLK * 2], dt.bfloat16, kind="ExternalInput")
    ivc_d = nc.dram_tensor("invcnt", [2, 1], dt.float32, kind="ExternalInput")
    cw_d = nc.dram_tensor("clfw", [2, 64], dt.float32, kind="ExternalInput")
    cb_d = nc.dram_tensor("clfb", [2, 1], dt.float32, kind="ExternalInput")
    GT_d, rr_d, br_d = {}, {}, {}
    for l, (ci, co) in enumerate(LAYERS, 1):
        d = ci * co
        GT_d[l] = nc.dram_tensor(f"GT{l}", [K, d], dt.bfloat16, kind="ExternalInput")
        rr_d[l] = nc.dram_tensor(f"rootrep{l}", [128, d], dt.bfloat16, kind="ExternalInput")
        br_d[l] = nc.dram_tensor(f"biasrep{l}", [128, co], dt.float32, kind="ExternalInput")
    out_d = nc.dram_tensor("out", [2, 1], dt.float32, kind="ExternalOutput")
    hdbg_d = nc.dram_tensor("hdbg", [128, 4 * NBLK * 64], dt.float32,
                            kind="ExternalOutput") if debug_h else None
    hsh_d = nc.dram_tensor("hshard", [8, NPC], dt.float32, kind="Internal")
    hfull_d = nc.dram_tensor("hfull", [NCORE * 8, NPC], dt.float32, kind="Internal",
                             addr_space="Shared")
    warm_in_d = nc.dram_tensor("warmin", [8, 16], dt.float32, kind="Internal")
    warm_out_d = nc.dram_tensor("warmout", [NCORE * 8, 16], dt.float32, kind="Internal",
                                addr_space="Shared")

    with tile.TileContext(nc) as tc:
        with tc.tile_pool(name="const", bufs=1) as cpool, \
             tc.tile_pool(name="stream", bufs=3) as spool, \
             tc.tile_pool(name="work", bufs=4) as wpool, \
             tc.tile_pool(name="ps_s", bufs=4, space="PSUM") as ps_s, \
             tc.tile_pool(name="ps_agg", bufs=2, space="PSUM") as ps_agg, \
             tc.tile_pool(name="ps_tr", bufs=2, space="PSUM") as ps_tr:

            # ---- collective-stream warmup (overlaps layer-1 compute) ----
            if not nocc and not nowarm:
                wt = cpool.tile([8, 16], dt.float32)
                nc.vector.memset(wt[:], 0.0)
                nc.sync.dma_start(out=warm_in_d[:], in_=wt[:])
                nc.gpsimd.collective_compute(
                    kind="AllGather", op=mybir.AluOpType.bypass,
                    replica_groups=[list(range(NCORE))],
                    ins=[warm_in_d[:]], outs=[warm_out_d[:]])

            # ---- critical-path preamble ---------------------------------
            GT, rr, br = {}, {}, {}
            for l, (ci, co) in enumerate(LAYERS, 1):
                d = ci * co
                GT[l] = cpool.tile([K, d], dt.bfloat16, tag=f"GT{l}", name=f"GT{l}t")
                nc.sync.dma_start(out=GT[l][:], in_=GT_d[l][:])
            gi = cpool.tile([128, T], dt.int16)
            nc.sync.dma_start(out=gi[:], in_=gi_d[:])
            # layer-1 gather table: one HBM read of xT, then on-chip replication
            table = cpool.tile([128, N, 1], dt.float32)
            nc.sync.dma_start(out=table[0:16, :, 0], in_=xT_d[:])
            for g in range(1, 8):
                nc.sync.dma_start(out=table[16 * g:16 * g + 16, :, 0],
                                  in_=table[0:16, :, 0])
            hloc = cpool.tile([128, NBLK, 16], dt.bfloat16)
            nc.sync.dma_start(out=hloc[:], in_=xl_d[:].rearrange("p (b i) -> p b i", i=16))
            d1 = cpool.tile([128, T * 64], dt.bfloat16)
            NCH = 5
            for c in range(NCH):
                c0, c1 = (T // NCH) * c * 64, (T // NCH) * (c + 1) * 64
                nc.sync.dma_start(out=d1[:, c0:c1], in_=d1_d[:, c0:c1])
            dg = cpool.tile([128, NBLK * 128], dt.bfloat16)
            nc.sync.dma_start(out=dg[:], in_=dg_d[:])
            ivd = cpool.tile([128, NBLK], dt.float32)
            nc.sync.dma_start(out=ivd[:], in_=ivd_d[:])
            for l in range(1, 5):
                d = LAYERS[l - 1][0] * LAYERS[l - 1][1]
                rr[l] = cpool.tile([128, d], dt.bfloat16, tag=f"rr{l}", name=f"rr{l}t")
                nc.sync.dma_start(out=rr[l][:], in_=rr_d[l][:])
                br[l] = cpool.tile([128, LAYERS[l - 1][1]], dt.float32, tag=f"br{l}",
                                   name=f"br{l}t")
                nc.sync.dma_start(out=br[l][:], in_=br_d[l][:])
            g2t = cpool.tile([128, NBLK * 2], dt.bfloat16)
            nc.sync.dma_start(out=g2t[:], in_=g2_d[:])
            h4 = cpool.tile([128, NBLK, 64], dt.bfloat16)
            hTsb = cpool.tile([16, NPC], dt.float32)
            ident = cpool.tile([128, 128], dt.float32)
            make_identity(nc, ident[:])
            identb = cpool.tile([128, 128], dt.bfloat16)
            make_identity(nc, identb[:])

            for l, (ci, co) in enumerate(LAYERS, 1):
                d = ci * co
                nts = max(1, 512 // d)        # message tiles per PSUM slot
                hdst = h4 if l == 4 else hloc
                # per-layer self-tiles: Vs_all[:, b, :] = rootrep * h_b (bcast over o)
                vsall = cpool.tile([128, NBLK, 512], dt.bfloat16, tag="vsall",
                                   name=f"vsall{l}")
                hb = hloc[:, 0, 0:ci]
                h_self = bass.AP(hb.tensor, hb.offset,
                                 [hb.ap[0], [16, NBLK], [0, co], [1, ci]])
                rr0 = rr[l][:]
                rr_b = bass.AP(rr0.tensor, rr0.offset,
                               [rr0.ap[0], [0, NBLK], [1, d]])
                nc.vector.tensor_tensor(
                    out=vsall[:, :, 0:d].rearrange("p b (o i) -> p b o i", i=ci),
                    in0=rr_b.rearrange("p b (o i) -> p b o i", i=ci),
                    in1=h_self, op=mybir.AluOpType.mult)

                # pre-gather h[src] with a 16-group rotating window (decouples
                # the gpsimd gather chain from the per-group pipeline)
                GW = 16
                htg_all = cpool.tile([128, GW * 128, 1], dt.float32, tag="htgall",
                                     name=f"htgall{l}")

                def emit_gather(qg):
                    if nogather:
                        return
                    nc.gpsimd.ap_gather(
                        out_ap=htg_all[:, (qg % GW) * 128:(qg % GW + 1) * 128, :],
                        in_ap=table[:],
                        idxs_ap=gi[:, qg * 8:(qg + 1) * 8],
                        channels=128, num_elems=N, d=1, num_idxs=128)

                if nogather:
                    nc.vector.memset(htg_all[:], 0.25)
                for q0 in range(min(GW, NG)):
                    emit_gather(q0)

                cur_agg = {}      # block -> (psum tile, [started_w0, started_w1])
                # prefetch one-hot chunks two groups ahead (issued AFTER the
                # consuming W-matmuls of the slot's previous tenant)
                ohcs = {}
                for q0 in range(min(2, NG)):
                    ohcs[q0] = spool.tile([K, 1024], dt.bfloat16, tag="oh",
                                          name=f"oh{l}_{q0}")
                    nc.sync.dma_start(out=ohcs[q0][:],
                                      in_=oh_d[:, q0 * 1024:(q0 + 1) * 1024])
                for q in range(NG):
                    ohc = ohcs.pop(q)
                    qw = (q % GW) * 128
                    if l == 4:
                        Ptr = ps_tr.tile([128, 128], dt.float32, tag="tr")
                        nc.tensor.transpose(out=Ptr[:],
                                            in_=htg_all[:, qw:qw + 128, 0],
                                            identity=ident[:])
                    else:
                        htgb = wpool.tile([128, 128], dt.bfloat16, tag="htgb")
                        nc.scalar.copy(out=htgb[:],
                                       in_=htg_all[:, qw:qw + 128, 0])
                        Ptr = ps_tr.tile([128, 128], dt.bfloat16, tag="tr")
                        nc.tensor.transpose(out=Ptr[:], in_=htgb[:], identity=identb[:])
                    htr = wpool.tile([128, 128], dt.bfloat16, tag="htr")
                    nc.scalar.copy(out=htr[:], in_=Ptr[:])
                    if q + GW < NG:
                        emit_gather(q + GW)

                    # A) all W matmuls of the group (keeps PE stream dense)
                    Pss = []
                    for s in range(8 // nts):
                        Ps = ps_s.tile([128, 512], dt.float32, tag="s",
                                       name=f"s{l}_{q}_{s}")
                        for t in range(nts):
                            g8 = s * nts + t
                            nc.tensor.matmul(out=Ps[:, t * d:(t + 1) * d],
                                             lhsT=ohc[:, g8 * 128:(g8 + 1) * 128],
                                             rhs=GT[l][:], start=True, stop=True)
                        Pss.append(Ps)
                    if q + 2 < NG:
                        ohcs[q + 2] = spool.tile([K, 1024], dt.bfloat16, tag="oh",
                                                 name=f"oh{l}_{q + 2}")
                        nc.sync.dma_start(out=ohcs[q + 2][:],
                                          in_=oh_d[:, (q + 2) * 1024:(q + 3) * 1024])
                    # B) V = relu(A) * h[src]
                    Vss = []
                    for s in range(8 // nts):
                        g80 = s * nts
                        h_in1 = bass.AP(htr.tensor, htr[:].offset + g80 * 16,
                                        [htr[:].ap[0], [16, nts], [0, co], [1, ci]])
                        V = wpool.tile([128, 512], dt.bfloat16, tag="V",
                                       name=f"V{l}_{q}_{s}")
                        if l in (2, 3):
                            # h >= 0 here, so relu(A)*h == relu(A*h): multiply
                            # straight out of PSUM, then a cheap 4x-mode relu.
                            V2 = wpool.tile([128, 512], dt.bfloat16, tag="W",
                                            name=f"V2{l}_{q}_{s}")
                            nc.vector.tensor_tensor(
                                out=V2[:].rearrange("p (t o i) -> p t o i", t=nts, i=ci),
                                in0=Pss[s][:].rearrange("p (t o i) -> p t o i",
                                                        t=nts, i=ci),
                                in1=h_in1, op=mybir.AluOpType.mult)
                            nc.vector.tensor_scalar(out=V[:], in0=V2[:],
                                                    scalar1=0.0, scalar2=None,
                                                    op0=mybir.AluOpType.max)
                        else:
                            # relu-evac on ScalarE, mult on DVE
                            Wsl = wpool.tile([128, 512], dt.bfloat16, tag="W",
                                             name=f"W{l}_{q}_{s}")
                            nc.scalar.activation(
                                out=Wsl[:], in_=Pss[s][:],
                                func=mybir.ActivationFunctionType.Relu)
                            nc.vector.tensor_tensor(
                                out=V[:].rearrange("p (t o i) -> p t o i", t=nts, i=ci),
                                in0=Wsl[:].rearrange("p (t o i) -> p t o i",
                                                     t=nts, i=ci),
                                in1=h_in1, op=mybir.AluOpType.mult)
                        Vss.append(V)
                    # C) scatter-accumulate per tile
                    for g8 in range(8):
                        s, t = g8 // nts, g8 % nts
                        tau = 8 * q + g8
                        b = tau // TB
                        w = 0 if (tau - b * TB) < TB // 2 else 1
                        if b not in cur_agg:
                            Pagg_new = ps_agg.tile([128, 512], dt.float32,
                                                   tag="agg", name=f"agg{l}_{b}")
                            cur_agg[b] = (Pagg_new, [False, False])
                        Pagg, started = cur_agg[b]
                        nc.tensor.matmul(out=Pagg[w * 64:(w + 1) * 64, 0:d],
                                         lhsT=d1[:, tau * 64:(tau + 1) * 64],
                                         rhs=Vss[s][:, t * d:(t + 1) * d],
                                         start=not started[w], stop=False)
                        started[w] = True
                        if tau == b * TB + TB - 1:
                            # ---- block tail: finish node update -----
                            nc.tensor.matmul(out=Pagg[:, 0:d],
                                             lhsT=dg[:, b * 128:(b + 1) * 128],
                                             rhs=vsall[:, b, 0:d],
                                             start=False, stop=True)
                            S = wpool.tile([128, co], dt.float32, tag="S",
                                           name=f"S{l}_{b}")
                            nc.vector.tensor_reduce(
                                out=S[:],
                                in_=Pagg[:, 0:d].rearrange("p (o i) -> p o i", i=ci),
                                axis=mybir.AxisListType.X, op=mybir.AluOpType.add)
                            S2 = wpool.tile([128, co], dt.float32, tag="S2",
                                            name=f"S2{l}_{b}")
                            nc.scalar.activation(out=S2[:], in_=S[:],
                                                 func=mybir.ActivationFunctionType.Copy,
                                                 scale=ivd[:, b:b + 1])
                            S3 = wpool.tile([128, co], dt.float32, tag="S3",
                                            name=f"S3{l}_{b}")
                            nc.vector.tensor_tensor(out=S3[:], in0=S2[:],
                                                    in1=br[l][:],
                                                    op=mybir.AluOpType.add)
                            S4 = wpool.tile([128, co], dt.float32, tag="S4",
                                            name=f"S4{l}_{b}")
                            nc.vector.tensor_scalar(out=S4[:], in0=S3[:],
                                                    scalar1=0.0, scalar2=None,
                                                    op0=mybir.AluOpType.max)
                            nc.vector.tensor_copy(out=hdst[:, b, 0:co], in_=S4[:])
                            if debug_h:
                                nc.sync.dma_start(
                                    out=hdbg_d[:][:, ((l - 1) * NBLK + b) * 64:
                                                  ((l - 1) * NBLK + b) * 64 + co],
                                    in_=S4[:])
                            del cur_agg[b]

                # ---- share h across cores, refill gather table ----------
                if l < 4:
                    # deferred transposes h[:, b, :co] -> hTsb (off the PE
                    # critical path during the group loop)
                    for b in range(NBLK):
                        Ptr2 = ps_tr.tile([128, 128], dt.bfloat16, tag="tr",
                                          name=f"tr{l}_{b}")
                        nc.tensor.transpose(out=Ptr2[0:co, 0:128],
                                            in_=hdst[:, b, 0:co],
                                            identity=identb[:])
                        nc.scalar.copy(out=hTsb[0:co, b * 128:(b + 1) * 128],
                                       in_=Ptr2[0:co, 0:128])
                    nc.sync.dma_start(out=hsh_d[:], in_=hTsb[0:8, :])
                    if not nocc:
                        nc.gpsimd.collective_compute(
                            kind="AllGather", op=mybir.AluOpType.bypass,
                            replica_groups=[list(range(NCORE))],
                            ins=[hsh_d[:]], outs=[hfull_d[:]])
                        hf = hfull_d[:]
                        src_ap = bass.AP(hf.tensor, 0, [[NPC, 8], [8 * NPC, 8], [1, NPC]])
                        for g in range(8):
                            nc.sync.dma_start(
                                out=table[16 * g:16 * g + 8, :, 0]
                                    .rearrange("p (c n) -> p c n", c=8),
                                in_=src_ap)

            # ---- pooling + classifier -----------------------------------
            Pp = ps_tr.tile([128, 128], dt.float32, tag="tr")
            for b in range(NBLK):
                nc.tensor.matmul(out=Pp[0:2, 0:64], lhsT=g2t[:, b * 2:(b + 1) * 2],
                                 rhs=h4[:, b, :], start=(b == 0), stop=(b == NBLK - 1))
            pool = wpool.tile([2, 64], dt.float32, tag="pool")
            ivc = cpool.tile([2, 1], dt.float32)
            nc.sync.dma_start(out=ivc[:], in_=ivc_d[:])
            cw = cpool.tile([2, 64], dt.float32)
            nc.sync.dma_start(out=cw[:], in_=cw_d[:])
            cb = cpool.tile([2, 1], dt.float32)
            nc.sync.dma_start(out=cb[:], in_=cb_d[:])
            nc.vector.tensor_scalar(out=pool[:], in0=Pp[0:2, 0:64], scalar1=ivc[:],
                                    scalar2=None, op0=mybir.AluOpType.mult)
            pz = wpool.tile([2, 64], dt.float32, tag="pz")
            nc.vector.tensor_tensor(out=pz[:], in0=pool[:], in1=cw[:], op=mybir.AluOpType.mult)
            z = wpool.tile([2, 1], dt.float32, tag="z")
            nc.vector.tensor_reduce(out=z[:], in_=pz[:], axis=mybir.AxisListType.X,
                                    op=mybir.AluOpType.add)
            z2 = wpool.tile([2, 1], dt.float32, tag="z2")
            nc.vector.tensor_tensor(out=z2[:], in0=z[:], in1=cb[:], op=mybir.AluOpType.add)
            z3 = wpool.tile([2, 1], dt.float32, tag="z3")
            nc.scalar.activation(out=z3[:], in_=z2[:],
                                 func=mybir.ActivationFunctionType.Sigmoid)
            nc.sync.dma_start(out=out_d[:], in_=z3[:])

    nc.compile()
    _CACHE[key] = nc
    return nc


def kernel(**inputs):
    debug_h = bool(os.environ.get("ATHENA_DEBUG_H"))
    TB, K, shared, per_core = _prep(inputs)
    nc = _build(TB, K, debug_h)
    in_maps = []
    for c in range(NCORE):
        m = dict(shared)
        m.update(per_core[c])
        in_maps.append(m)
    res = run_bass_kernel_spmd(nc, in_maps, core_ids=list(range(NCORE)),
                               trace=bool(os.environ.get("ATHENA_TRACE")))
    kernel.last_results = res
    outs = [res.results[c]["out"] for c in range(NCORE)]
    return np.concatenate(outs, axis=0).astype(np.float32)


# revision 27
# speedup vs baseline: 1.3446x; 1.0391x over previous
"""CellSpatialNet (4-layer NNConv GNN) on 8 trn2 NeuronCores.

Strategy: shard nodes+edges by dst across 8 cores (2560 nodes = 2 graphs/core).
Host folds the EdgeNN into per-type tables G0/G1(/G2) [36, d] so that
  W[e] = relu(ef0*G0[t_e] + ef1*G1[t_e] (+ G2[t_e]))
becomes ONE PE matmul per 128-edge tile with a host-built "scaled one-hot"
stationary operand [K, 128] (K=72 when the bias tables are zero, 108 else).
h[src] is fetched with ap_gather (free-axis SBUF gather) from a transposed,
group-replicated h-table, then PE-transposed back to edge-on-partition
layout.  relu+mult are FUSED into one DVE scalar_tensor_tensor per PSUM
slot: V = max(A,0)*h_bcast.  Scatter-mean is a PE matmul with a dst-one-hot
stationary accumulated in PSUM per 128-node block, expanded over (o,i) and
reduced on DVE.  Root/bias terms ride as per-layer precomputed self-tiles.
Between layers a [8, 2560] fp32 AllGather shares h; a dummy AllGather at
kernel start warms the collective stream concurrently with layer-1 compute.
"""
import os
import numpy as np
import ml_dtypes

import concourse.bass as bass
from concourse import bacc
import concourse.mybir as mybir
import concourse.tile as tile
from concourse.bass_utils import run_bass_kernel_spmd
from concourse.masks import make_identity

BF16 = ml_dtypes.bfloat16

N, E, B = 20480, 327680, 16
ET, EF = 36, 2
NCORE = 8
NPC = N // NCORE        # 2560 nodes per core
NBLK = NPC // 128       # 20 node blocks per core
GPC = B // NCORE        # 2 graphs per core
NPG = N // B            # 1280 nodes per graph
LAYERS = [(16, 8), (8, 8), (8, 8), (8, 64)]


def _oi_perm(ci, co):
    """column permutation taking (i,o)-flat [d] -> (o,i)-flat [d]."""
    k = np.arange(ci * co)
    o, i = k // ci, k % ci
    return i * co + o  # new[k=(o,i)] = old[i*co+o]


def _prep(inputs):
    """All host-side numpy preprocessing. Returns (TB, K, shared, per_core)."""
    x = np.asarray(inputs["x"], np.float32)
    ei = np.asarray(inputs["edge_index"], np.int64)
    etype = np.asarray(inputs["edge_type"], np.int64)
    ea = np.asarray(inputs["edge_attr"], np.float32)
    ct = np.asarray(inputs["cell_type"], np.int64)
    src, dst = ei[0], ei[1]

    deg = np.bincount(dst, minlength=N).astype(np.float32)
    inv_deg = 1.0 / np.maximum(deg, 1.0)

    shared = {"xT": np.ascontiguousarray(x.T).astype(np.float32)}
    g2zero = True
    Gts = {}
    for l, (ci, co) in enumerate(LAYERS, 1):
        d = ci * co
        emb = np.asarray(inputs[f"emb{l}"], np.float32)
        wh = np.asarray(inputs[f"wh{l}"], np.float32)
        bh = np.asarray(inputs[f"bh{l}"], np.float32)
        wg = np.asarray(inputs[f"wg{l}"], np.float32)
        bg = np.asarray(inputs[f"bg{l}"], np.float32)
        root = np.asarray(inputs[f"root{l}"], np.float32)
        bias = np.asarray(inputs[f"bias{l}"], np.float32)
        G0 = emb * wh[0][None, :] + np.broadcast_to(wg[0], (ET, d))
        G1 = emb * wh[1][None, :] + np.broadcast_to(wg[1], (ET, d))
        G2 = emb * bh[None, :] + np.broadcast_to(bg, (ET, d))
        if np.abs(G2).max() > 0:
            g2zero = False
        p = _oi_perm(ci, co)
        Gts[l] = (G0[:, p], G1[:, p], G2[:, p])
        rr = root.reshape(ci, co).T.reshape(-1)  # (o,i) flat: rr[o*ci+i] = root[i,o]
        shared[f"rootrep{l}"] = np.broadcast_to(rr, (128, d)).astype(BF16).copy()
        shared[f"biasrep{l}"] = np.broadcast_to(bias, (128, co)).astype(np.float32).copy()
    K = 2 * ET if g2zero else 3 * ET
    for l, (ci, co) in enumerate(LAYERS, 1):
        G0p, G1p, G2p = Gts[l]
        parts = [G0p, G1p] if g2zero else [G0p, G1p, G2p]
        shared[f"GT{l}"] = np.concatenate(parts, axis=0).astype(BF16)  # [K, d]

    clf_w = np.asarray(inputs["clf_w"], np.float32)   # [64, 1]
    clf_b = np.asarray(inputs["clf_b"], np.float32)   # [1]
    shared["clfw"] = np.broadcast_to(clf_w[:, 0], (2, 64)).astype(np.float32).copy()
    shared["clfb"] = np.full((2, 1), clf_b[0], np.float32)

    # replication matrices for building the gather table on the PE:
    # table[16g+f, :] <- stage[f, :]   (startup, stage=xT [16, N])
    rep16 = np.zeros((16, 128), np.float32)
    for g in range(8):
        rep16[np.arange(16), 16 * g + np.arange(16)] = 1.0
    shared["rep16"] = rep16
    # table[16g+f, c*NPC+j] <- hfs[8c+f, j]   (per-core-column block c)
    rep64 = np.zeros((64, 8 * 128), np.float32)
    for c in range(8):
        for f in range(8):
            for g in range(8):
                rep64[8 * c + f, c * 128 + 16 * g + f] = 1.0
    shared["rep64"] = rep64

    # ---- per-core edge organization -------------------------------------
    # window w of an edge = (dst_local_in_block)//64; per (core, block, window)
    # the edges form ceil(n/128) tiles; TB/2 tiles allocated per window.
    per_core_ew = []  # [(core)][block][window] -> array of edge ids
    halfmax = 1
    for c in range(NCORE):
        lo = c * NPC
        em = np.where((dst >= lo) & (dst < lo + NPC))[0]
        dl = dst[em] - lo
        order = np.argsort(dl, kind="stable")
        em, dl = em[order], dl[order]
        blocks = []
        for b in range(NBLK):
            sel = (dl // 128) == b
            ebm, dbm = em[sel], dl[sel]
            wsel = ((dbm % 128) // 64) == 0
            blocks.append((ebm[wsel], ebm[~wsel]))
            halfmax = max(halfmax, -(-len(ebm[wsel]) // 128), -(-len(ebm[~wsel]) // 128))
        per_core_ew.append(blocks)
    TB = 2 * halfmax        # tiles per block (even), window w gets tiles [w*TB/2,(w+1)*TB/2)
    T = NBLK * TB           # message tiles per core per layer
    assert T % 8 == 0

    per_core = []
    for c in range(NCORE):
        lo = c * NPC
        oh = np.zeros((K, T * 128), BF16)
        d1 = np.zeros((128, T * 64), BF16)
        dgd = np.zeros((128, NBLK * 128), BF16)
        gidx = np.zeros((128, T), np.int16)
        for b in range(NBLK):
            for w in (0, 1):
                edges = per_core_ew[c][b][w]
                for t in range(TB // 2):
                    tau = b * TB + w * (TB // 2) + t
                    seg = edges[t * 128:(t + 1) * 128]
                    n = len(seg)
                    if n:
                        p = np.arange(n)
                        tt = etype[seg]
                        cols = tau * 128 + p
                        oh[tt, cols] = ea[seg, 0].astype(BF16)
                        oh[ET + tt, cols] = ea[seg, 1].astype(BF16)
                        if K == 3 * ET:
                            oh[2 * ET + tt, cols] = BF16(1.0)
                        d1[p, tau * 64 + (dst[seg] - lo - b * 128 - w * 64)] = BF16(1.0)
                    # gather indices (wrapped per 16 partitions within group g=tau%8)
                    g = tau % 8
                    q = tau // 8
                    j = np.arange(128)
                    srcs = np.zeros(128, np.int16)
                    srcs[:n] = src[seg].astype(np.int16)
                    gidx[16 * g + (j % 16), q * 8 + j // 16] = srcs
            pb = np.arange(128)
            dgd[pb, b * 128 + pb] = deg[lo + b * 128 + pb].astype(BF16)

        xl = np.zeros((128, NBLK * 16), BF16)
        ivd = np.zeros((128, NBLK), np.float32)
        g2 = np.zeros((128, NBLK * 2), BF16)
        for b in range(NBLK):
            nodes = lo + b * 128 + np.arange(128)
            xl[:, b * 16:(b + 1) * 16] = x[nodes].astype(BF16)
            ivd[:, b] = inv_deg[nodes]
            g2[:, b * 2 + (b >= 10)] = (ct[nodes] == 1).astype(np.float32).astype(BF16)
        cnt = np.array([[(ct[lo:lo + NPG] == 1).sum()], [(ct[lo + NPG:lo + NPC] == 1).sum()]], np.float32)
        ivc = 1.0 / np.maximum(cnt, 1.0)
        per_core.append({"onehotS": oh, "dst1h": d1, "dstdiag": dgd, "gidx": gidx,
                         "xloc": xl, "invdeg": ivd, "gate2": g2, "invcnt": ivc})
    return TB, K, shared, per_core


_CACHE = {}


def _build(TB, K, debug_h=False):
    nocc = bool(os.environ.get("ATHENA_NOCC"))
    nogather = bool(os.environ.get("ATHENA_NOGATHER"))
    nowarm = bool(os.environ.get("ATHENA_NOWARM"))
    key = (TB, K, debug_h, nocc, nogather, nowarm)
    if key in _CACHE:
        return _CACHE[key]
    T = NBLK * TB
    NG = T // 8                       # 8-tile groups per layer
    dt = mybir.dt
    nc = bacc.Bacc("TRN2", target_bir_lowering=False, num_devices=NCORE)

    xT_d = nc.dram_tensor("xT", [16, N], dt.float32, kind="ExternalInput")
    xl_d = nc.dram_tensor("xloc", [128, NBLK * 16], dt.bfloat16, kind="ExternalInput")
    oh_d = nc.dram_tensor("onehotS", [K, T * 128], dt.bfloat16, kind="ExternalInput")
    d1_d = nc.dram_tensor("dst1h", [128, T * 64], dt.bfloat16, kind="ExternalInput")
    dg_d = nc.dram_tensor("dstdiag", [128, NBLK * 128], dt.bfloat16, kind="ExternalInput")
    gi_d = nc.dram_tensor("gidx", [128, T], dt.int16, kind="ExternalInput")
    ivd_d = nc.dram_tensor("invdeg", [128, NBLK], dt.float32, kind="ExternalInput")
    g2_d = nc.dram_tensor("gate2", [128, NBLK * 2], dt.bfloat16, kind="ExternalInput")
    ivc_d = nc.dram_tensor("invcnt", [2, 1], dt.float32, kind="ExternalInput")
    cw_d = nc.dram_tensor("clfw", [2, 64], dt.float32, kind="ExternalInput")
    cb_d = nc.dram_tensor("clfb", [2, 1], dt.float32, kind="ExternalInput")
    GT_d, rr_d, br_d = {}, {}, {}
    for l, (ci, co) in enumerate(LAYERS, 1):
        d = ci * co
        GT_d[l] = nc.dram_tensor(f"GT{l}", [K, d], dt.bfloat16, kind="ExternalInput")
        rr_d[l] = nc.dram_tensor(f"rootrep{l}", [128, d], dt.bfloat16, kind="ExternalInput")
        br_d[l] = nc.dram_tensor(f"biasrep{l}", [128, co], dt.float32, kind="ExternalInput")
    out_d = nc.dram_tensor("out", [2, 1], dt.float32, kind="ExternalOutput")
    hdbg_d = nc.dram_tensor("hdbg", [128, 4 * NBLK * 64], dt.float32,
                            kind="ExternalOutput") if debug_h else None
    hsh_d = nc.dram_tensor("hshard", [8, NPC], dt.float32, kind="Internal")
    hfull_d = nc.dram_tensor("hfull", [NCORE * 8, NPC], dt.float32, kind="Internal",
                             addr_space="Shared")
    r16_d = nc.dram_tensor("rep16", [16, 128], dt.float32, kind="ExternalInput")
    r64_d = nc.dram_tensor("rep64", [64, 8 * 128], dt.float32, kind="ExternalInput")
    warm_in_d = nc.dram_tensor("warmin", [8, 16], dt.float32, kind="Internal")
    warm_out_d = nc.dram_tensor("warmout", [NCORE * 8, 16], dt.float32, kind="Internal",
                                addr_space="Shared")

    with tile.TileContext(nc) as tc:
        with tc.tile_pool(name="const", bufs=1) as cpool, \
             tc.tile_pool(name="stream", bufs=5) as spool, \
             tc.tile_pool(name="stg", bufs=1) as stgpool, \
             tc.tile_pool(name="work", bufs=4) as wpool, \
             tc.tile_pool(name="ps_s", bufs=4, space="PSUM") as ps_s, \
             tc.tile_pool(name="ps_agg", bufs=2, space="PSUM") as ps_agg, \
             tc.tile_pool(name="ps_tr", bufs=2, space="PSUM") as ps_tr:

            # ---- collective-stream warmup (overlaps layer-1 compute) ----
            if not nocc and not nowarm:
                wt = cpool.tile([8, 16], dt.float32)
                nc.vector.memset(wt[:], 0.0)
                nc.sync.dma_start(out=warm_in_d[:], in_=wt[:])
                nc.gpsimd.collective_compute(
                    kind="AllGather", op=mybir.AluOpType.bypass,
                    replica_groups=[list(range(NCORE))],
                    ins=[warm_in_d[:]], outs=[warm_out_d[:]])

            # ---- critical-path preamble ---------------------------------
            GT, rr, br = {}, {}, {}
            for l, (ci, co) in enumerate(LAYERS, 1):
                d = ci * co
                GT[l] = cpool.tile([K, d], dt.bfloat16, tag=f"GT{l}", name=f"GT{l}t")
                nc.sync.dma_start(out=GT[l][:], in_=GT_d[l][:])
            gi = cpool.tile([128, T], dt.int16)
            nc.sync.dma_start(out=gi[:], in_=gi_d[:])
            rep16 = cpool.tile([16, 128], dt.float32)
            nc.sync.dma_start(out=rep16[:], in_=r16_d[:])
            rep64 = cpool.tile([64, 8 * 128], dt.float32)
            nc.sync.dma_start(out=rep64[:], in_=r64_d[:])
            hloc = cpool.tile([128, NBLK, 16], dt.bfloat16)
            nc.sync.dma_start(out=hloc[:], in_=xl_d[:].rearrange("p (b i) -> p b i", i=16))

            # layer-1 gather table, built via PE replication matmuls:
            # stage a column-chunk of xT in SBUF, matmul with rep16, evacuate.
            table = cpool.tile([128, N, 1], dt.float32)
            CW = 2560                      # stage chunk (fp32 columns)
            n_ch = N // CW                 # 8 chunks
            for ch in range(n_ch):
                stg = stgpool.tile([64, CW], dt.float32, tag="stage",
                                   name=f"xstg{ch}")
                nc.sync.dma_start(out=stg[0:16, :],
                                  in_=xT_d[:, ch * CW:(ch + 1) * CW])
                for sc in range(CW // 512):
                    Pt = ps_s.tile([128, 512], dt.float32, tag="s",
                                   name=f"xrep{ch}_{sc}")
                    nc.tensor.matmul(out=Pt[:], lhsT=rep16[:],
                                     rhs=stg[0:16, sc * 512:(sc + 1) * 512],
                                     start=True, stop=True)
                    dst = table[:, ch * CW + sc * 512:ch * CW + (sc + 1) * 512, 0]
                    if sc % 2 == 0:
                        nc.vector.tensor_copy(out=dst, in_=Pt[:])
                    else:
                        nc.scalar.copy(out=dst, in_=Pt[:])

            d1 = cpool.tile([128, T * 64], dt.bfloat16)
            NCH = 12
            d1_bounds = [(T * c // NCH) * 64 for c in range(NCH + 1)]
            d1_next = [0]

            def emit_d1_chunk():
                c = d1_next[0]
                if c < NCH:
                    nc.sync.dma_start(out=d1[:, d1_bounds[c]:d1_bounds[c + 1]],
                                      in_=d1_d[:, d1_bounds[c]:d1_bounds[c + 1]])
                    d1_next[0] = c + 1
            emit_d1_chunk()
            emit_d1_chunk()
            dg = cpool.tile([128, NBLK * 128], dt.bfloat16)
            nc.sync.dma_start(out=dg[:], in_=dg_d[:])
            ivd = cpool.tile([128, NBLK], dt.float32)
            nc.sync.dma_start(out=ivd[:], in_=ivd_d[:])
            for l in range(1, 5):
                d = LAYERS[l - 1][0] * LAYERS[l - 1][1]
                rr[l] = cpool.tile([128, d], dt.bfloat16, tag=f"rr{l}", name=f"rr{l}t")
                nc.sync.dma_start(out=rr[l][:], in_=rr_d[l][:])
                br[l] = cpool.tile([128, LAYERS[l - 1][1]], dt.float32, tag=f"br{l}",
                                   name=f"br{l}t")
                nc.sync.dma_start(out=br[l][:], in_=br_d[l][:])
            g2t = cpool.tile([128, NBLK * 2], dt.bfloat16)
            nc.sync.dma_start(out=g2t[:], in_=g2_d[:])
            h4 = cpool.tile([128, NBLK, 64], dt.bfloat16)
            ident = cpool.tile([128, 128], dt.float32)
            make_identity(nc, ident[:])
            identb = cpool.tile([128, 128], dt.bfloat16)
            make_identity(nc, identb[:])

            for l, (ci, co) in enumerate(LAYERS, 1):
                d = ci * co
                nts = max(1, 512 // d)        # message tiles per PSUM slot
                hdst = h4 if l == 4 else hloc
                # per-layer self-tiles: Vs_all[:, b, :] = rootrep * h_b (bcast over o)
                vsall = cpool.tile([128, NBLK, 512], dt.bfloat16, tag="vsall",
                                   name=f"vsall{l}")
                hb = hloc[:, 0, 0:ci]
                h_self = bass.AP(hb.tensor, hb.offset,
                                 [hb.ap[0], [16, NBLK], [0, co], [1, ci]])
                rr0 = rr[l][:]
                rr_b = bass.AP(rr0.tensor, rr0.offset,
                               [rr0.ap[0], [0, NBLK], [1, d]])
                nc.vector.tensor_tensor(
                    out=vsall[:, :, 0:d].rearrange("p b (o i) -> p b o i", i=ci),
                    in0=rr_b.rearrange("p b (o i) -> p b o i", i=ci),
                    in1=h_self, op=mybir.AluOpType.mult)

                # pre-gather h[src] with a 16-group rotating window (decouples
                # the gpsimd gather chain from the per-group pipeline)
                GW = 12
                htg_all = cpool.tile([128, GW * 128, 1], dt.float32, tag="htgall",
                                     name=f"htgall{l}")

                def emit_gather(qg):
                    if nogather:
                        return
                    nc.gpsimd.ap_gather(
                        out_ap=htg_all[:, (qg % GW) * 128:(qg % GW + 1) * 128, :],
                        in_ap=table[:],
                        idxs_ap=gi[:, qg * 8:(qg + 1) * 8],
                        channels=128, num_elems=N, d=1, num_idxs=128)

                if nogather:
                    nc.vector.memset(htg_all[:], 0.25)
                for q0 in range(min(GW, NG)):
                    emit_gather(q0)

                cur_agg = {}      # block -> (psum tile, [started_w0, started_w1])
                # prefetch one-hot chunks PF groups ahead, each split into two
                # row-half DMAs so transfers spread across DMA queues
                PF = 4
                KH = K // 2

                def emit_oh(qo):
                    t_ = spool.tile([K, 1024], dt.bfloat16, tag="oh",
                                    name=f"oh{l}_{qo}")
                    nc.sync.dma_start(out=t_[0:KH, :],
                                      in_=oh_d[0:KH, qo * 1024:(qo + 1) * 1024])
                    nc.sync.dma_start(out=t_[KH:K, :],
                                      in_=oh_d[KH:K, qo * 1024:(qo + 1) * 1024])
                    return t_

                ohcs = {}
                for q0 in range(min(PF, NG)):
                    ohcs[q0] = emit_oh(q0)
                for q in range(NG):
                    ohc = ohcs.pop(q)
                    # A) all W matmuls of the group first (PE FIFO not blocked
                    # on the gather/transpose path)
                    Pss = []
                    for s in range(8 // nts):
                        Ps = ps_s.tile([128, 512], dt.float32, tag="s",
                                       name=f"s{l}_{q}_{s}")
                        for t in range(nts):
                            g8 = s * nts + t
                            nc.tensor.matmul(out=Ps[:, t * d:(t + 1) * d],
                                             lhsT=ohc[:, g8 * 128:(g8 + 1) * 128],
                                             rhs=GT[l][:], start=True, stop=True)
                        Pss.append(Ps)
                    if q + PF < NG:
                        ohcs[q + PF] = emit_oh(q + PF)
                    if l == 1 and q % 4 == 1:
                        emit_d1_chunk()
                    qw = (q % GW) * 128
                    if l == 4:
                        Ptr = ps_tr.tile([128, 128], dt.float32, tag="tr")
                        nc.tensor.transpose(out=Ptr[:],
                                            in_=htg_all[:, qw:qw + 128, 0],
                                            identity=ident[:])
                    else:
                        htgb = wpool.tile([128, 128], dt.bfloat16, tag="htgb")
                        nc.scalar.copy(out=htgb[:],
                                       in_=htg_all[:, qw:qw + 128, 0])
                        Ptr = ps_tr.tile([128, 128], dt.bfloat16, tag="tr")
                        nc.tensor.transpose(out=Ptr[:], in_=htgb[:], identity=identb[:])
                    htr = wpool.tile([128, 128], dt.bfloat16, tag="htr")
                    nc.scalar.copy(out=htr[:], in_=Ptr[:])
                    if q + GW < NG:
                        emit_gather(q + GW)
                    # B) V = relu(A) * h[src]
                    Vss = []
                    for s in range(8 // nts):
                        g80 = s * nts
                        h_in1 = bass.AP(htr.tensor, htr[:].offset + g80 * 16,
                                        [htr[:].ap[0], [16, nts], [0, co], [1, ci]])
                        V = wpool.tile([128, 512], dt.bfloat16, tag="V",
                                       name=f"V{l}_{q}_{s}")
                        if l in (2, 3):
                            # h >= 0 here, so relu(A)*h == relu(A*h): multiply
                            # straight out of PSUM, then a cheap 4x-mode relu.
                            V2 = wpool.tile([128, 512], dt.bfloat16, tag="W",
                                            name=f"V2{l}_{q}_{s}")
                            nc.vector.tensor_tensor(
                                out=V2[:].rearrange("p (t o i) -> p t o i", t=nts, i=ci),
                                in0=Pss[s][:].rearrange("p (t o i) -> p t o i",
                                                        t=nts, i=ci),
                                in1=h_in1, op=mybir.AluOpType.mult)
                            nc.vector.tensor_scalar(out=V[:], in0=V2[:],
                                                    scalar1=0.0, scalar2=None,
                                                    op0=mybir.AluOpType.max)
                        else:
                            # relu-evac on ScalarE, mult on DVE
                            Wsl = wpool.tile([128, 512], dt.bfloat16, tag="W",
                                             name=f"W{l}_{q}_{s}")
                            nc.scalar.activation(
                                out=Wsl[:], in_=Pss[s][:],
                                func=mybir.ActivationFunctionType.Relu)
                            nc.vector.tensor_tensor(
                                out=V[:].rearrange("p (t o i) -> p t o i", t=nts, i=ci),
                                in0=Wsl[:].rearrange("p (t o i) -> p t o i",
                                                     t=nts, i=ci),
                                in1=h_in1, op=mybir.AluOpType.mult)
                        Vss.append(V)
                    # C) scatter-accumulate per tile
                    for g8 in range(8):
                        s, t = g8 // nts, g8 % nts
                        tau = 8 * q + g8
                        b = tau // TB
                        w = 0 if (tau - b * TB) < TB // 2 else 1
                        if b not in cur_agg:
                            Pagg_new = ps_agg.tile([128, 512], dt.float32,
                                                   tag="agg", name=f"agg{l}_{b}")
                            cur_agg[b] = (Pagg_new, [False, False])
                        Pagg, started = cur_agg[b]
                        nc.tensor.matmul(out=Pagg[w * 64:(w + 1) * 64, 0:d],
                                         lhsT=d1[:, tau * 64:(tau + 1) * 64],
                                         rhs=Vss[s][:, t * d:(t + 1) * d],
                                         start=not started[w], stop=False)
                        started[w] = True
                        if tau == b * TB + TB - 1:
                            # ---- block tail: finish node update -----
                            nc.tensor.matmul(out=Pagg[:, 0:d],
                                             lhsT=dg[:, b * 128:(b + 1) * 128],
                                             rhs=vsall[:, b, 0:d],
                                             start=False, stop=True)
                            S = wpool.tile([128, co], dt.float32, tag="S",
                                           name=f"S{l}_{b}")
                            nc.vector.tensor_reduce(
                                out=S[:],
                                in_=Pagg[:, 0:d].rearrange("p (o i) -> p o i", i=ci),
                                axis=mybir.AxisListType.X, op=mybir.AluOpType.add)
                            S2 = wpool.tile([128, co], dt.float32, tag="S2",
                                            name=f"S2{l}_{b}")
                            nc.scalar.activation(out=S2[:], in_=S[:],
                                                 func=mybir.ActivationFunctionType.Copy,
                                                 scale=ivd[:, b:b + 1])
                            S3 = wpool.tile([128, co], dt.float32, tag="S3",
                                            name=f"S3{l}_{b}")
                            nc.vector.tensor_tensor(out=S3[:], in0=S2[:],
                                                    in1=br[l][:],
                                                    op=mybir.AluOpType.add)
                            S4 = wpool.tile([128, co], dt.float32, tag="S4",
                                            name=f"S4{l}_{b}")
                            nc.vector.tensor_scalar(out=S4[:], in0=S3[:],
                                                    scalar1=0.0, scalar2=None,
                                                    op0=mybir.AluOpType.max)
                            nc.vector.tensor_copy(out=hdst[:, b, 0:co], in_=S4[:])
                            if debug_h:
                                nc.sync.dma_start(
                                    out=hdbg_d[:][:, ((l - 1) * NBLK + b) * 64:
                                                  ((l - 1) * NBLK + b) * 64 + co],
                                    in_=S4[:])
                            del cur_agg[b]

                # ---- share h across cores, refill gather table ----------
                if l < 4:
                    # deferred transposes h[:, b, :co] -> hTsb (off the PE
                    # critical path during the group loop)
                    for b in range(NBLK):
                        Ptr2 = ps_tr.tile([128, 128], dt.bfloat16, tag="tr",
                                          name=f"tr{l}_{b}")
                        nc.tensor.transpose(out=Ptr2[0:co, 0:128],
                                            in_=hdst[:, b, 0:co],
                                            identity=identb[:])
                        hTs = wpool.tile([8, 128], dt.float32, tag="hTs",
                                         name=f"hTs{l}_{b}")
                        nc.scalar.copy(out=hTs[:], in_=Ptr2[0:co, 0:128])
                        nc.sync.dma_start(out=hsh_d[:, b * 128:(b + 1) * 128],
                                          in_=hTs[:])
                    if not nocc:
                        nc.gpsimd.collective_compute(
                            kind="AllGather", op=mybir.AluOpType.bypass,
                            replica_groups=[list(range(NCORE))],
                            ins=[hsh_d[:]], outs=[hfull_d[:]])
                        # stage hfull into SBUF with many small DMAs (spread
                        # across queues), then rebuild the gather table with
                        # PE replication matmuls (DMA fabric here is slow)
                        hfs = stgpool.tile([64, NPC], dt.float32, tag="stage",
                                           name=f"hfs{l}")
                        CQ = NPC // 4
                        for rg in range(4):
                            for cq in range(4):
                                nc.sync.dma_start(
                                    out=hfs[16 * rg:16 * rg + 16,
                                            cq * CQ:(cq + 1) * CQ],
                                    in_=hfull_d[16 * rg:16 * rg + 16,
                                                cq * CQ:(cq + 1) * CQ])
                        for c in range(8):
                            for sc in range(NPC // 512):
                                Pt = ps_s.tile([128, 512], dt.float32, tag="s",
                                               name=f"hrep{l}_{c}_{sc}")
                                nc.tensor.matmul(
                                    out=Pt[:], lhsT=rep64[:, c * 128:(c + 1) * 128],
                                    rhs=hfs[:, sc * 512:(sc + 1) * 512],
                                    start=True, stop=True)
                                dst = table[:, c * NPC + sc * 512:
                                            c * NPC + (sc + 1) * 512, 0]
                                if sc % 2 == 0:
                                    nc.vector.tensor_copy(out=dst, in_=Pt[:])
                                else:
                                    nc.scalar.copy(out=dst, in_=Pt[:])

            # ---- pooling + classifier -----------------------------------
            Pp = ps_tr.tile([128, 128], dt.float32, tag="tr")
            for b in range(NBLK):
                nc.tensor.matmul(out=Pp[0:2, 0:64], lhsT=g2t[:, b * 2:(b + 1) * 2],
                                 rhs=h4[:, b, :], start=(b == 0), stop=(b == NBLK - 1))
            pool = wpool.tile([2, 64], dt.float32, tag="pool")
            ivc = cpool.tile([2, 1], dt.float32)
            nc.sync.dma_start(out=ivc[:], in_=ivc_d[:])
            cw = cpool.tile([2, 64], dt.float32)
            nc.sync.dma_start(out=cw[:], in_=cw_d[:])
            cb = cpool.tile([2, 1], dt.float32)
            nc.sync.dma_start(out=cb[:], in_=cb_d[:])
            nc.vector.tensor_scalar(out=pool[:], in0=Pp[0:2, 0:64], scalar1=ivc[:],
                                    scalar2=None, op0=mybir.AluOpType.mult)
            pz = wpool.tile([2, 64], dt.float32, tag="pz")
            nc.vector.tensor_tensor(out=pz[:], in0=pool[:], in1=cw[:], op=mybir.AluOpType.mult)
            z = wpool.tile([2, 1], dt.float32, tag="z")
            nc.vector.tensor_reduce(out=z[:], in_=pz[:], axis=mybir.AxisListType.X,
                                    op=mybir.AluOpType.add)
            z2 = wpool.tile([2, 1], dt.float32, tag="z2")
            nc.vector.tensor_tensor(out=z2[:], in0=z[:], in1=cb[:], op=mybir.AluOpType.add)
            z3 = wpool.tile([2, 1], dt.float32, tag="z3")
            nc.scalar.activation(out=z3[:], in_=z2[:],
                                 func=mybir.ActivationFunctionType.Sigmoid)
            nc.sync.dma_start(out=out_d[:], in_=z3[:])

    nc.compile()
    _CACHE[key] = nc
    return nc


def kernel(**inputs):
    debug_h = bool(os.environ.get("ATHENA_DEBUG_H"))
    TB, K, shared, per_core = _prep(inputs)
    nc = _build(TB, K, debug_h)
    in_maps = []
    for c in range(NCORE):
        m = dict(shared)
        m.update(per_core[c])
        in_maps.append(m)
    res = run_bass_kernel_spmd(nc, in_maps, core_ids=list(range(NCORE)),
                               trace=bool(os.environ.get("ATHENA_TRACE")))
    kernel.last_results = res
    outs = [res.results[c]["out"] for c in range(NCORE)]
    return np.concatenate(outs, axis=0).astype(np.float32)


# revision 36
# speedup vs baseline: 1.4840x; 1.1037x over previous
"""CellSpatialNet (4-layer NNConv GNN) on 8 trn2 NeuronCores.

Strategy: shard nodes+edges by dst across 8 cores (2560 nodes = 2 graphs/core).
Host folds the EdgeNN into per-type tables G0/G1(/G2) [36, d] so that
  W[e] = relu(ef0*G0[t_e] + ef1*G1[t_e] (+ G2[t_e]))
becomes ONE PE matmul per 128-edge tile with a host-built "scaled one-hot"
stationary operand [K, 128] (K=72 when the bias tables are zero, 108 else).
h[src] is fetched with ap_gather (free-axis SBUF gather) from a transposed,
group-replicated h-table, then PE-transposed back to edge-on-partition
layout.  relu+mult are FUSED into one DVE scalar_tensor_tensor per PSUM
slot: V = max(A,0)*h_bcast.  Scatter-mean is a PE matmul with a dst-one-hot
stationary accumulated in PSUM per 128-node block, expanded over (o,i) and
reduced on DVE.  Root/bias terms ride as per-layer precomputed self-tiles.
Between layers a [8, 2560] fp32 AllGather shares h; a dummy AllGather at
kernel start warms the collective stream concurrently with layer-1 compute.
"""
import os
import numpy as np
import ml_dtypes

import concourse.bass as bass
from concourse import bacc
import concourse.mybir as mybir
import concourse.tile as tile
from concourse.bass_utils import run_bass_kernel_spmd
from concourse.masks import make_identity

BF16 = ml_dtypes.bfloat16

N, E, B = 20480, 327680, 16
ET, EF = 36, 2
NCORE = 8
NPC = N // NCORE        # 2560 nodes per core
NBLK = NPC // 128       # 20 node blocks per core
GPC = B // NCORE        # 2 graphs per core
NPG = N // B            # 1280 nodes per graph
LAYERS = [(16, 8), (8, 8), (8, 8), (8, 64)]


def _oi_perm(ci, co):
    """column permutation taking (i,o)-flat [d] -> (o,i)-flat [d]."""
    k = np.arange(ci * co)
    o, i = k // ci, k % ci
    return i * co + o  # new[k=(o,i)] = old[i*co+o]


def _prep(inputs):
    """All host-side numpy preprocessing. Returns (TB, K, shared, per_core)."""
    x = np.asarray(inputs["x"], np.float32)
    ei = np.asarray(inputs["edge_index"], np.int64)
    etype = np.asarray(inputs["edge_type"], np.int64)
    ea = np.asarray(inputs["edge_attr"], np.float32)
    ct = np.asarray(inputs["cell_type"], np.int64)
    src, dst = ei[0], ei[1]

    deg = np.bincount(dst, minlength=N).astype(np.float32)
    inv_deg = 1.0 / np.maximum(deg, 1.0)

    shared = {"xT": np.ascontiguousarray(x.T).astype(np.float32)}
    g2zero = True
    Gts = {}
    for l, (ci, co) in enumerate(LAYERS, 1):
        d = ci * co
        emb = np.asarray(inputs[f"emb{l}"], np.float32)
        wh = np.asarray(inputs[f"wh{l}"], np.float32)
        bh = np.asarray(inputs[f"bh{l}"], np.float32)
        wg = np.asarray(inputs[f"wg{l}"], np.float32)
        bg = np.asarray(inputs[f"bg{l}"], np.float32)
        root = np.asarray(inputs[f"root{l}"], np.float32)
        bias = np.asarray(inputs[f"bias{l}"], np.float32)
        G0 = emb * wh[0][None, :] + np.broadcast_to(wg[0], (ET, d))
        G1 = emb * wh[1][None, :] + np.broadcast_to(wg[1], (ET, d))
        G2 = emb * bh[None, :] + np.broadcast_to(bg, (ET, d))
        if np.abs(G2).max() > 0:
            g2zero = False
        p = _oi_perm(ci, co)
        Gts[l] = (G0[:, p], G1[:, p], G2[:, p])
        rr = root.reshape(ci, co).T.reshape(-1)  # (o,i) flat: rr[o*ci+i] = root[i,o]
        shared[f"rootrep{l}"] = np.broadcast_to(rr, (128, d)).astype(BF16).copy()
        shared[f"biasrep{l}"] = np.broadcast_to(bias, (128, co)).astype(np.float32).copy()
    K = 2 * ET if g2zero else 3 * ET
    for l, (ci, co) in enumerate(LAYERS, 1):
        G0p, G1p, G2p = Gts[l]
        parts = [G0p, G1p] if g2zero else [G0p, G1p, G2p]
        shared[f"GT{l}"] = np.concatenate(parts, axis=0).astype(BF16)  # [K, d]

    clf_w = np.asarray(inputs["clf_w"], np.float32)   # [64, 1]
    clf_b = np.asarray(inputs["clf_b"], np.float32)   # [1]
    shared["clfw"] = np.broadcast_to(clf_w[:, 0], (2, 64)).astype(np.float32).copy()
    shared["clfb"] = np.full((2, 1), clf_b[0], np.float32)

    # replication matrices for building the gather table on the PE:
    # table[16g+f, :] <- stage[f, :]   (startup, stage=xT [16, N])
    rep16 = np.zeros((16, 128), np.float32)
    for g in range(8):
        rep16[np.arange(16), 16 * g + np.arange(16)] = 1.0
    shared["rep16"] = rep16
    # table[16g+f, c*NPC+j] <- hfs[8c+f, j]   (per-core-column block c)
    rep64 = np.zeros((64, 8 * 128), np.float32)
    for c in range(8):
        for f in range(8):
            for g in range(8):
                rep64[8 * c + f, c * 128 + 16 * g + f] = 1.0
    shared["rep64"] = rep64

    # ---- per-core edge organization -------------------------------------
    # window w of an edge = (dst_local_in_block)//64; per (core, block, window)
    # the edges form ceil(n/128) tiles; TB/2 tiles allocated per window.
    per_core_ew = []  # [(core)][block][window] -> array of edge ids
    halfmax = 1
    for c in range(NCORE):
        lo = c * NPC
        em = np.where((dst >= lo) & (dst < lo + NPC))[0]
        dl = dst[em] - lo
        order = np.argsort(dl, kind="stable")
        em, dl = em[order], dl[order]
        blocks = []
        for b in range(NBLK):
            sel = (dl // 128) == b
            ebm, dbm = em[sel], dl[sel]
            wsel = ((dbm % 128) // 64) == 0
            blocks.append((ebm[wsel], ebm[~wsel]))
            halfmax = max(halfmax, -(-len(ebm[wsel]) // 128), -(-len(ebm[~wsel]) // 128))
        per_core_ew.append(blocks)
    TB = 2 * halfmax        # tiles per block (even), window w gets tiles [w*TB/2,(w+1)*TB/2)
    T = NBLK * TB           # message tiles per core per layer
    assert T % 8 == 0

    per_core = []
    for c in range(NCORE):
        lo = c * NPC
        oh = np.zeros((K, T * 128), BF16)
        d1 = np.zeros((128, T * 64), BF16)
        dgd = np.zeros((128, NBLK * 128), BF16)
        gidx = np.zeros((128, T), np.int16)
        xg = np.zeros((128, T * 16), BF16)   # layer-1 h[src], edge-ordered
        for b in range(NBLK):
            for w in (0, 1):
                edges = per_core_ew[c][b][w]
                for t in range(TB // 2):
                    tau = b * TB + w * (TB // 2) + t
                    seg = edges[t * 128:(t + 1) * 128]
                    n = len(seg)
                    if n:
                        p = np.arange(n)
                        tt = etype[seg]
                        cols = tau * 128 + p
                        oh[tt, cols] = ea[seg, 0].astype(BF16)
                        oh[ET + tt, cols] = ea[seg, 1].astype(BF16)
                        if K == 3 * ET:
                            oh[2 * ET + tt, cols] = BF16(1.0)
                        d1[p, tau * 64 + (dst[seg] - lo - b * 128 - w * 64)] = BF16(1.0)
                    # gather indices (wrapped per 16 partitions within group g=tau%8)
                    g = tau % 8
                    q = tau // 8
                    j = np.arange(128)
                    srcs = np.zeros(128, np.int16)
                    srcs[:n] = src[seg].astype(np.int16)
                    gidx[16 * g + (j % 16), q * 8 + j // 16] = srcs
                    # layer-1 gathered x in final [edge, 16g+i] layout
                    xg[:, (q * 8 + g) * 16:(q * 8 + g) * 16 + 16] = \
                        x[srcs.astype(np.int64)].astype(BF16)
            pb = np.arange(128)
            dgd[pb, b * 128 + pb] = deg[lo + b * 128 + pb].astype(BF16)

        xl = np.zeros((128, NBLK * 16), BF16)
        ivd = np.zeros((128, NBLK), np.float32)
        g2 = np.zeros((128, NBLK * 2), BF16)
        for b in range(NBLK):
            nodes = lo + b * 128 + np.arange(128)
            xl[:, b * 16:(b + 1) * 16] = x[nodes].astype(BF16)
            ivd[:, b] = inv_deg[nodes]
            g2[:, b * 2 + (b >= 10)] = (ct[nodes] == 1).astype(np.float32).astype(BF16)
        cnt = np.array([[(ct[lo:lo + NPG] == 1).sum()], [(ct[lo + NPG:lo + NPC] == 1).sum()]], np.float32)
        ivc = 1.0 / np.maximum(cnt, 1.0)
        per_core.append({"onehotS": oh, "dst1h": d1, "dstdiag": dgd, "gidx": gidx,
                         "xgath": xg, "xloc": xl, "invdeg": ivd, "gate2": g2,
                         "invcnt": ivc})
    return TB, K, shared, per_core


_CACHE = {}


def _build(TB, K, debug_h=False):
    nocc = bool(os.environ.get("ATHENA_NOCC"))
    nogather = bool(os.environ.get("ATHENA_NOGATHER"))
    nowarm = bool(os.environ.get("ATHENA_NOWARM"))
    key = (TB, K, debug_h, nocc, nogather, nowarm)
    if key in _CACHE:
        return _CACHE[key]
    T = NBLK * TB
    NG = T // 8                       # 8-tile groups per layer
    dt = mybir.dt
    nc = bacc.Bacc("TRN2", target_bir_lowering=False, num_devices=NCORE)

    xT_d = nc.dram_tensor("xT", [16, N], dt.float32, kind="ExternalInput")
    xl_d = nc.dram_tensor("xloc", [128, NBLK * 16], dt.bfloat16, kind="ExternalInput")
    oh_d = nc.dram_tensor("onehotS", [K, T * 128], dt.bfloat16, kind="ExternalInput")
    d1_d = nc.dram_tensor("dst1h", [128, T * 64], dt.bfloat16, kind="ExternalInput")
    dg_d = nc.dram_tensor("dstdiag", [128, NBLK * 128], dt.bfloat16, kind="ExternalInput")
    gi_d = nc.dram_tensor("gidx", [128, T], dt.int16, kind="ExternalInput")
    xg_d = nc.dram_tensor("xgath", [128, T * 16], dt.bfloat16, kind="ExternalInput")
    ivd_d = nc.dram_tensor("invdeg", [128, NBLK], dt.float32, kind="ExternalInput")
    g2_d = nc.dram_tensor("gate2", [128, NBLK * 2], dt.bfloat16, kind="ExternalInput")
    ivc_d = nc.dram_tensor("invcnt", [2, 1], dt.float32, kind="ExternalInput")
    cw_d = nc.dram_tensor("clfw", [2, 64], dt.float32, kind="ExternalInput")
    cb_d = nc.dram_tensor("clfb", [2, 1], dt.float32, kind="ExternalInput")
    GT_d, rr_d, br_d = {}, {}, {}
    for l, (ci, co) in enumerate(LAYERS, 1):
        d = ci * co
        GT_d[l] = nc.dram_tensor(f"GT{l}", [K, d], dt.bfloat16, kind="ExternalInput")
        rr_d[l] = nc.dram_tensor(f"rootrep{l}", [128, d], dt.bfloat16, kind="ExternalInput")
        br_d[l] = nc.dram_tensor(f"biasrep{l}", [128, co], dt.float32, kind="ExternalInput")
    out_d = nc.dram_tensor("out", [2, 1], dt.float32, kind="ExternalOutput")
    hdbg_d = nc.dram_tensor("hdbg", [128, 4 * NBLK * 64], dt.float32,
                            kind="ExternalOutput") if debug_h else None
    hsh_d = nc.dram_tensor("hshard", [8, NPC], dt.float32, kind="Internal")
    hfull_d = nc.dram_tensor("hfull", [NCORE * 8, NPC], dt.float32, kind="Internal",
                             addr_space="Shared")
    r16_d = nc.dram_tensor("rep16", [16, 128], dt.float32, kind="ExternalInput")
    r64_d = nc.dram_tensor("rep64", [64, 8 * 128], dt.float32, kind="ExternalInput")
    warm_in_d = nc.dram_tensor("warmin", [8, 16], dt.float32, kind="Internal")
    warm_out_d = nc.dram_tensor("warmout", [NCORE * 8, 16], dt.float32, kind="Internal",
                                addr_space="Shared")

    with tile.TileContext(nc) as tc:
        with tc.tile_pool(name="const", bufs=1) as cpool, \
             tc.tile_pool(name="stream", bufs=5) as spool, \
             tc.tile_pool(name="stg", bufs=1) as stgpool, \
             tc.tile_pool(name="work", bufs=4) as wpool, \
             tc.tile_pool(name="ps_s", bufs=4, space="PSUM") as ps_s, \
             tc.tile_pool(name="ps_agg", bufs=2, space="PSUM") as ps_agg, \
             tc.tile_pool(name="ps_tr", bufs=2, space="PSUM") as ps_tr:

            # ---- collective-stream warmup (overlaps layer-1 compute) ----
            if not nocc and not nowarm:
                wt = cpool.tile([8, 16], dt.float32)
                nc.vector.memset(wt[:], 0.0)
                nc.sync.dma_start(out=warm_in_d[:], in_=wt[:])
                nc.gpsimd.collective_compute(
                    kind="AllGather", op=mybir.AluOpType.bypass,
                    replica_groups=[list(range(NCORE))],
                    ins=[warm_in_d[:]], outs=[warm_out_d[:]])

            # ---- critical-path preamble ---------------------------------
            GT, rr, br = {}, {}, {}
            for l, (ci, co) in enumerate(LAYERS, 1):
                d = ci * co
                GT[l] = cpool.tile([K, d], dt.bfloat16, tag=f"GT{l}", name=f"GT{l}t")
                nc.sync.dma_start(out=GT[l][:], in_=GT_d[l][:])
            gi = cpool.tile([128, T], dt.int16)
            nc.sync.dma_start(out=gi[:], in_=gi_d[:])
            rep64 = cpool.tile([64, 8 * 128], dt.float32)
            nc.sync.dma_start(out=rep64[:], in_=r64_d[:])
            hloc = cpool.tile([128, NBLK, 16], dt.bfloat16)
            nc.sync.dma_start(out=hloc[:], in_=xl_d[:].rearrange("p (b i) -> p b i", i=16))

            # gather table for layers 2-4 (rebuilt at every layer boundary)
            table = cpool.tile([128, N, 1], dt.float32)
            d1 = cpool.tile([128, T * 64], dt.bfloat16)
            NCH = 12
            d1_bounds = [(T * c // NCH) * 64 for c in range(NCH + 1)]
            d1_next = [0]

            def emit_d1_chunk():
                c = d1_next[0]
                if c < NCH:
                    nc.sync.dma_start(out=d1[:, d1_bounds[c]:d1_bounds[c + 1]],
                                      in_=d1_d[:, d1_bounds[c]:d1_bounds[c + 1]])
                    d1_next[0] = c + 1
            emit_d1_chunk()
            emit_d1_chunk()
            dg = cpool.tile([128, NBLK * 128], dt.bfloat16)
            nc.sync.dma_start(out=dg[:], in_=dg_d[:])
            ivd = cpool.tile([128, NBLK], dt.float32)
            nc.sync.dma_start(out=ivd[:], in_=ivd_d[:])
            for l in range(1, 5):
                d = LAYERS[l - 1][0] * LAYERS[l - 1][1]
                rr[l] = cpool.tile([128, d], dt.bfloat16, tag=f"rr{l}", name=f"rr{l}t")
                nc.sync.dma_start(out=rr[l][:], in_=rr_d[l][:])
                br[l] = cpool.tile([128, LAYERS[l - 1][1]], dt.float32, tag=f"br{l}",
                                   name=f"br{l}t")
                nc.sync.dma_start(out=br[l][:], in_=br_d[l][:])
            g2t = cpool.tile([128, NBLK * 2], dt.bfloat16)
            nc.sync.dma_start(out=g2t[:], in_=g2_d[:])
            h4 = cpool.tile([128, NBLK, 64], dt.bfloat16)
            ident = cpool.tile([128, 128], dt.float32)
            make_identity(nc, ident[:])
            identb = cpool.tile([128, 128], dt.bfloat16)
            make_identity(nc, identb[:])

            for l, (ci, co) in enumerate(LAYERS, 1):
                d = ci * co
                nts = max(1, 512 // d)        # message tiles per PSUM slot
                hdst = h4 if l == 4 else hloc
                # per-layer self-tiles: Vs_all[:, b, :] = rootrep * h_b (bcast over o)
                vsall = cpool.tile([128, NBLK, 512], dt.bfloat16, tag="vsall",
                                   name=f"vsall{l}")
                hb = hloc[:, 0, 0:ci]
                h_self = bass.AP(hb.tensor, hb.offset,
                                 [hb.ap[0], [16, NBLK], [0, co], [1, ci]])
                rr0 = rr[l][:]
                rr_b = bass.AP(rr0.tensor, rr0.offset,
                               [rr0.ap[0], [0, NBLK], [1, d]])
                nc.vector.tensor_tensor(
                    out=vsall[:, :, 0:d].rearrange("p b (o i) -> p b o i", i=ci),
                    in0=rr_b.rearrange("p b (o i) -> p b o i", i=ci),
                    in1=h_self, op=mybir.AluOpType.mult)

                # pre-gather h[src] with a rotating window (decouples the
                # gpsimd gather chain from the per-group pipeline); layer 1
                # instead streams host-precomputed x[src] straight from HBM
                GW = 11
                if l > 1:
                    htg_all = cpool.tile([128, GW * 128, 1], dt.float32,
                                         tag="htgall", name=f"htgall{l}")

                def emit_gather(qg):
                    if nogather or l == 1:
                        return
                    nc.gpsimd.ap_gather(
                        out_ap=htg_all[:, (qg % GW) * 128:(qg % GW + 1) * 128, :],
                        in_ap=table[:],
                        idxs_ap=gi[:, qg * 8:(qg + 1) * 8],
                        channels=128, num_elems=N, d=1, num_idxs=128)

                if nogather and l > 1:
                    nc.vector.memset(htg_all[:], 0.25)
                for q0 in range(min(GW, NG)):
                    emit_gather(q0)

                cur_agg = {}      # block -> (psum tile, [started_w0, started_w1])
                # prefetch one-hot chunks PF groups ahead, each split into two
                # row-half DMAs so transfers spread across DMA queues
                PF = 4
                KH = K // 2

                def emit_oh(qo):
                    t_ = spool.tile([K, 1024], dt.bfloat16, tag="oh",
                                    name=f"oh{l}_{qo}")
                    nc.sync.dma_start(out=t_[0:KH, :],
                                      in_=oh_d[0:KH, qo * 1024:(qo + 1) * 1024])
                    nc.sync.dma_start(out=t_[KH:K, :],
                                      in_=oh_d[KH:K, qo * 1024:(qo + 1) * 1024])
                    return t_

                def emit_xg(qo):
                    t_ = spool.tile([128, 128], dt.bfloat16, tag="xg",
                                    name=f"xg_{qo}")
                    nc.sync.dma_start(out=t_[:],
                                      in_=xg_d[:, qo * 128:(qo + 1) * 128])
                    return t_

                ohcs, xgs = {}, {}
                for q0 in range(min(PF, NG)):
                    ohcs[q0] = emit_oh(q0)
                    if l == 1:
                        xgs[q0] = emit_xg(q0)
                for q in range(NG):
                    ohc = ohcs.pop(q)
                    # A) all W matmuls of the group first (PE FIFO not blocked
                    # on the gather/transpose path)
                    Pss = []
                    for s in range(8 // nts):
                        Ps = ps_s.tile([128, 512], dt.float32, tag="s",
                                       name=f"s{l}_{q}_{s}")
                        for t in range(nts):
                            g8 = s * nts + t
                            nc.tensor.matmul(out=Ps[:, t * d:(t + 1) * d],
                                             lhsT=ohc[:, g8 * 128:(g8 + 1) * 128],
                                             rhs=GT[l][:], start=True, stop=True)
                        Pss.append(Ps)
                    if q + PF < NG:
                        ohcs[q + PF] = emit_oh(q + PF)
                        if l == 1:
                            xgs[q + PF] = emit_xg(q + PF)
                    if l == 1 and q % 4 == 1:
                        emit_d1_chunk()
                    qw = (q % GW) * 128
                    if l == 1:
                        htr = xgs.pop(q)
                    elif l == 4:
                        Ptr = ps_tr.tile([128, 128], dt.float32, tag="tr")
                        nc.tensor.transpose(out=Ptr[:],
                                            in_=htg_all[:, qw:qw + 128, 0],
                                            identity=ident[:])
                        htr = wpool.tile([128, 128], dt.bfloat16, tag="htr")
                        nc.scalar.copy(out=htr[:], in_=Ptr[:])
                    else:
                        htgb = wpool.tile([128, 128], dt.bfloat16, tag="htgb")
                        nc.scalar.copy(out=htgb[:],
                                       in_=htg_all[:, qw:qw + 128, 0])
                        Ptr = ps_tr.tile([128, 128], dt.bfloat16, tag="tr")
                        nc.tensor.transpose(out=Ptr[:], in_=htgb[:], identity=identb[:])
                        htr = wpool.tile([128, 128], dt.bfloat16, tag="htr")
                        nc.scalar.copy(out=htr[:], in_=Ptr[:])
                    if q + GW < NG:
                        emit_gather(q + GW)
                    # B) V = relu(A) * h[src]
                    Vss = []
                    for s in range(8 // nts):
                        g80 = s * nts
                        h_in1 = bass.AP(htr.tensor, htr[:].offset + g80 * 16,
                                        [htr[:].ap[0], [16, nts], [0, co], [1, ci]])
                        V = wpool.tile([128, 512], dt.bfloat16, tag="V",
                                       name=f"V{l}_{q}_{s}")
                        if l in (2, 3):
                            # h >= 0 here, so relu(A)*h == relu(A*h): multiply
                            # straight out of PSUM, then a cheap 4x-mode relu.
                            V2 = wpool.tile([128, 512], dt.bfloat16, tag="W",
                                            name=f"V2{l}_{q}_{s}")
                            nc.vector.tensor_tensor(
                                out=V2[:].rearrange("p (t o i) -> p t o i", t=nts, i=ci),
                                in0=Pss[s][:].rearrange("p (t o i) -> p t o i",
                                                        t=nts, i=ci),
                                in1=h_in1, op=mybir.AluOpType.mult)
                            nc.vector.tensor_scalar(out=V[:], in0=V2[:],
                                                    scalar1=0.0, scalar2=None,
                                                    op0=mybir.AluOpType.max)
                        else:
                            # relu-evac on ScalarE, mult on DVE
                            Wsl = wpool.tile([128, 512], dt.bfloat16, tag="W",
                                             name=f"W{l}_{q}_{s}")
                            nc.scalar.activation(
                                out=Wsl[:], in_=Pss[s][:],
                                func=mybir.ActivationFunctionType.Relu)
                            nc.vector.tensor_tensor(
                                out=V[:].rearrange("p (t o i) -> p t o i", t=nts, i=ci),
                                in0=Wsl[:].rearrange("p (t o i) -> p t o i",
                                                     t=nts, i=ci),
                                in1=h_in1, op=mybir.AluOpType.mult)
                        Vss.append(V)
                    # C) scatter-accumulate per tile
                    for g8 in range(8):
                        s, t = g8 // nts, g8 % nts
                        tau = 8 * q + g8
                        b = tau // TB
                        w = 0 if (tau - b * TB) < TB // 2 else 1
                        if b not in cur_agg:
                            Pagg_new = ps_agg.tile([128, 512], dt.float32,
                                                   tag="agg", name=f"agg{l}_{b}")
                            cur_agg[b] = (Pagg_new, [False, False])
                        Pagg, started = cur_agg[b]
                        nc.tensor.matmul(out=Pagg[w * 64:(w + 1) * 64, 0:d],
                                         lhsT=d1[:, tau * 64:(tau + 1) * 64],
                                         rhs=Vss[s][:, t * d:(t + 1) * d],
                                         start=not started[w], stop=False)
                        started[w] = True
                        if tau == b * TB + TB - 1:
                            # ---- block tail: finish node update -----
                            nc.tensor.matmul(out=Pagg[:, 0:d],
                                             lhsT=dg[:, b * 128:(b + 1) * 128],
                                             rhs=vsall[:, b, 0:d],
                                             start=False, stop=True)
                            S = wpool.tile([128, co], dt.float32, tag="S",
                                           name=f"S{l}_{b}")
                            nc.vector.tensor_reduce(
                                out=S[:],
                                in_=Pagg[:, 0:d].rearrange("p (o i) -> p o i", i=ci),
                                axis=mybir.AxisListType.X, op=mybir.AluOpType.add)
                            S2 = wpool.tile([128, co], dt.float32, tag="S2",
                                            name=f"S2{l}_{b}")
                            nc.scalar.activation(out=S2[:], in_=S[:],
                                                 func=mybir.ActivationFunctionType.Copy,
                                                 scale=ivd[:, b:b + 1])
                            S3 = wpool.tile([128, co], dt.float32, tag="S3",
                                            name=f"S3{l}_{b}")
                            nc.vector.tensor_tensor(out=S3[:], in0=S2[:],
                                                    in1=br[l][:],
                                                    op=mybir.AluOpType.add)
                            S4 = wpool.tile([128, co], dt.float32, tag="S4",
                                            name=f"S4{l}_{b}")
                            nc.vector.tensor_scalar(out=S4[:], in0=S3[:],
                                                    scalar1=0.0, scalar2=None,
                                                    op0=mybir.AluOpType.max)
                            nc.vector.tensor_copy(out=hdst[:, b, 0:co], in_=S4[:])
                            if debug_h:
                                nc.sync.dma_start(
                                    out=hdbg_d[:][:, ((l - 1) * NBLK + b) * 64:
                                                  ((l - 1) * NBLK + b) * 64 + co],
                                    in_=S4[:])
                            del cur_agg[b]

                # ---- share h across cores, refill gather table ----------
                if l < 4:
                    # deferred transposes h[:, b, :co] -> hTsb (off the PE
                    # critical path during the group loop)
                    for b in range(NBLK):
                        Ptr2 = ps_tr.tile([128, 128], dt.bfloat16, tag="tr",
                                          name=f"tr{l}_{b}")
                        nc.tensor.transpose(out=Ptr2[0:co, 0:128],
                                            in_=hdst[:, b, 0:co],
                                            identity=identb[:])
                        hTs = wpool.tile([8, 128], dt.float32, tag="hTs",
                                         name=f"hTs{l}_{b}")
                        nc.scalar.copy(out=hTs[:], in_=Ptr2[0:co, 0:128])
                        nc.sync.dma_start(out=hsh_d[:, b * 128:(b + 1) * 128],
                                          in_=hTs[:])
                    if not nocc:
                        nc.gpsimd.collective_compute(
                            kind="AllGather", op=mybir.AluOpType.bypass,
                            replica_groups=[list(range(NCORE))],
                            ins=[hsh_d[:]], outs=[hfull_d[:]])
                        # stage hfull into SBUF with many small DMAs (spread
                        # across queues), then rebuild the gather table with
                        # PE replication matmuls (DMA fabric here is slow)
                        hfs = stgpool.tile([64, NPC], dt.float32, tag="stage",
                                           name=f"hfs{l}")
                        CQ = NPC // 4
                        for rg in range(4):
                            for cq in range(4):
                                nc.sync.dma_start(
                                    out=hfs[16 * rg:16 * rg + 16,
                                            cq * CQ:(cq + 1) * CQ],
                                    in_=hfull_d[16 * rg:16 * rg + 16,
                                                cq * CQ:(cq + 1) * CQ])
                        for c in range(8):
                            for sc in range(NPC // 512):
                                Pt = ps_s.tile([128, 512], dt.float32, tag="s",
                                               name=f"hrep{l}_{c}_{sc}")
                                nc.tensor.matmul(
                                    out=Pt[:], lhsT=rep64[:, c * 128:(c + 1) * 128],
                                    rhs=hfs[:, sc * 512:(sc + 1) * 512],
                                    start=True, stop=True)
                                dst = table[:, c * NPC + sc * 512:
                                            c * NPC + (sc + 1) * 512, 0]
                                if sc % 2 == 0:
                                    nc.vector.tensor_copy(out=dst, in_=Pt[:])
                                else:
                                    nc.scalar.copy(out=dst, in_=Pt[:])

            # ---- pooling + classifier -----------------------------------
            Pp = ps_tr.tile([128, 128], dt.float32, tag="tr")
            for b in range(NBLK):
                nc.tensor.matmul(out=Pp[0:2, 0:64], lhsT=g2t[:, b * 2:(b + 1) * 2],
                                 rhs=h4[:, b, :], start=(b == 0), stop=(b == NBLK - 1))
            pool = wpool.tile([2, 64], dt.float32, tag="pool")
            ivc = cpool.tile([2, 1], dt.float32)
            nc.sync.dma_start(out=ivc[:], in_=ivc_d[:])
            cw = cpool.tile([2, 64], dt.float32)
            nc.sync.dma_start(out=cw[:], in_=cw_d[:])
            cb = cpool.tile([2, 1], dt.float32)
            nc.sync.dma_start(out=cb[:], in_=cb_d[:])
            nc.vector.tensor_scalar(out=pool[:], in0=Pp[0:2, 0:64], scalar1=ivc[:],
                                    scalar2=None, op0=mybir.AluOpType.mult)
            pz = wpool.tile([2, 64], dt.float32, tag="pz")
            nc.vector.tensor_tensor(out=pz[:], in0=pool[:], in1=cw[:], op=mybir.AluOpType.mult)
            z = wpool.tile([2, 1], dt.float32, tag="z")
            nc.vector.tensor_reduce(out=z[:], in_=pz[:], axis=mybir.AxisListType.X,
                                    op=mybir.AluOpType.add)
            z2 = wpool.tile([2, 1], dt.float32, tag="z2")
            nc.vector.tensor_tensor(out=z2[:], in0=z[:], in1=cb[:], op=mybir.AluOpType.add)
            z3 = wpool.tile([2, 1], dt.float32, tag="z3")
            nc.scalar.activation(out=z3[:], in_=z2[:],
                                 func=mybir.ActivationFunctionType.Sigmoid)
            nc.sync.dma_start(out=out_d[:], in_=z3[:])

    nc.compile()
    _CACHE[key] = nc
    return nc


def kernel(**inputs):
    debug_h = bool(os.environ.get("ATHENA_DEBUG_H"))
    TB, K, shared, per_core = _prep(inputs)
    nc = _build(TB, K, debug_h)
    in_maps = []
    for c in range(NCORE):
        m = dict(shared)
        m.update(per_core[c])
        in_maps.append(m)
    res = run_bass_kernel_spmd(nc, in_maps, core_ids=list(range(NCORE)),
                               trace=bool(os.environ.get("ATHENA_TRACE")))
    kernel.last_results = res
    outs = [res.results[c]["out"] for c in range(NCORE)]
    return np.concatenate(outs, axis=0).astype(np.float32)


# revision 37
# speedup vs baseline: 1.6937x; 1.1413x over previous
"""CellSpatialNet (4-layer NNConv GNN) on 8 trn2 NeuronCores.

Strategy: shard nodes+edges by dst across 8 cores (2560 nodes = 2 graphs/core).
Host folds the EdgeNN into per-type tables G0/G1(/G2) [36, d] so that
  W[e] = relu(ef0*G0[t_e] + ef1*G1[t_e] (+ G2[t_e]))
becomes ONE PE matmul per 128-edge tile with a host-built "scaled one-hot"
stationary operand [K, 128] (K=72 when the bias tables are zero, 108 else).
h[src] is fetched with ap_gather (free-axis SBUF gather) from a transposed,
group-replicated h-table, then PE-transposed back to edge-on-partition
layout.  relu+mult are FUSED into one DVE scalar_tensor_tensor per PSUM
slot: V = max(A,0)*h_bcast.  Scatter-mean is a PE matmul with a dst-one-hot
stationary accumulated in PSUM per 128-node block, expanded over (o,i) and
reduced on DVE.  Root/bias terms ride as per-layer precomputed self-tiles.
Between layers a [8, 2560] fp32 AllGather shares h; a dummy AllGather at
kernel start warms the collective stream concurrently with layer-1 compute.
"""
import os
import numpy as np
import ml_dtypes

import concourse.bass as bass
from concourse import bacc
import concourse.mybir as mybir
import concourse.tile as tile
from concourse.bass_utils import run_bass_kernel_spmd
from concourse.masks import make_identity

BF16 = ml_dtypes.bfloat16

N, E, B = 20480, 327680, 16
ET, EF = 36, 2
NCORE = 8
NPC = N // NCORE        # 2560 nodes per core
NBLK = NPC // 128       # 20 node blocks per core
GPC = B // NCORE        # 2 graphs per core
NPG = N // B            # 1280 nodes per graph
LAYERS = [(16, 8), (8, 8), (8, 8), (8, 64)]


def _oi_perm(ci, co):
    """column permutation taking (i,o)-flat [d] -> (o,i)-flat [d]."""
    k = np.arange(ci * co)
    o, i = k // ci, k % ci
    return i * co + o  # new[k=(o,i)] = old[i*co+o]


def _prep(inputs):
    """All host-side numpy preprocessing. Returns (TB, K, shared, per_core)."""
    x = np.asarray(inputs["x"], np.float32)
    ei = np.asarray(inputs["edge_index"], np.int64)
    etype = np.asarray(inputs["edge_type"], np.int64)
    ea = np.asarray(inputs["edge_attr"], np.float32)
    ct = np.asarray(inputs["cell_type"], np.int64)
    src, dst = ei[0], ei[1]

    deg = np.bincount(dst, minlength=N).astype(np.float32)
    inv_deg = 1.0 / np.maximum(deg, 1.0)

    shared = {"xT": np.ascontiguousarray(x.T).astype(np.float32)}
    g2zero = True
    Gts = {}
    for l, (ci, co) in enumerate(LAYERS, 1):
        d = ci * co
        emb = np.asarray(inputs[f"emb{l}"], np.float32)
        wh = np.asarray(inputs[f"wh{l}"], np.float32)
        bh = np.asarray(inputs[f"bh{l}"], np.float32)
        wg = np.asarray(inputs[f"wg{l}"], np.float32)
        bg = np.asarray(inputs[f"bg{l}"], np.float32)
        root = np.asarray(inputs[f"root{l}"], np.float32)
        bias = np.asarray(inputs[f"bias{l}"], np.float32)
        G0 = emb * wh[0][None, :] + np.broadcast_to(wg[0], (ET, d))
        G1 = emb * wh[1][None, :] + np.broadcast_to(wg[1], (ET, d))
        G2 = emb * bh[None, :] + np.broadcast_to(bg, (ET, d))
        if np.abs(G2).max() > 0:
            g2zero = False
        p = _oi_perm(ci, co)
        Gts[l] = (G0[:, p], G1[:, p], G2[:, p])
        rr = root.reshape(ci, co).T.reshape(-1)  # (o,i) flat: rr[o*ci+i] = root[i,o]
        shared[f"rootrep{l}"] = np.broadcast_to(rr, (128, d)).astype(BF16).copy()
        shared[f"biasrep{l}"] = np.broadcast_to(bias, (128, co)).astype(np.float32).copy()
    K = 2 * ET if g2zero else 3 * ET
    for l, (ci, co) in enumerate(LAYERS, 1):
        G0p, G1p, G2p = Gts[l]
        parts = [G0p, G1p] if g2zero else [G0p, G1p, G2p]
        shared[f"GT{l}"] = np.concatenate(parts, axis=0).astype(BF16)  # [K, d]

    clf_w = np.asarray(inputs["clf_w"], np.float32)   # [64, 1]
    clf_b = np.asarray(inputs["clf_b"], np.float32)   # [1]
    shared["clfw"] = np.broadcast_to(clf_w[:, 0], (2, 64)).astype(np.float32).copy()
    shared["clfb"] = np.full((2, 1), clf_b[0], np.float32)

    # replication matrices for building the gather table on the PE:
    # table[16g+f, :] <- stage[f, :]   (startup, stage=xT [16, N])
    rep16 = np.zeros((16, 128), np.float32)
    for g in range(8):
        rep16[np.arange(16), 16 * g + np.arange(16)] = 1.0
    shared["rep16"] = rep16
    # table[16g+f, c*NPC+j] <- hfs[8c+f, j]   (per-core-column block c)
    rep64 = np.zeros((64, 8 * 128), np.float32)
    for c in range(8):
        for f in range(8):
            for g in range(8):
                rep64[8 * c + f, c * 128 + 16 * g + f] = 1.0
    shared["rep64"] = rep64.astype(BF16)

    # ---- per-core edge organization -------------------------------------
    # window w of an edge = (dst_local_in_block)//64; per (core, block, window)
    # the edges form ceil(n/128) tiles; TB/2 tiles allocated per window.
    per_core_ew = []  # [(core)][block][window] -> array of edge ids
    halfmax = 1
    for c in range(NCORE):
        lo = c * NPC
        em = np.where((dst >= lo) & (dst < lo + NPC))[0]
        dl = dst[em] - lo
        order = np.argsort(dl, kind="stable")
        em, dl = em[order], dl[order]
        blocks = []
        for b in range(NBLK):
            sel = (dl // 128) == b
            ebm, dbm = em[sel], dl[sel]
            wsel = ((dbm % 128) // 64) == 0
            blocks.append((ebm[wsel], ebm[~wsel]))
            halfmax = max(halfmax, -(-len(ebm[wsel]) // 128), -(-len(ebm[~wsel]) // 128))
        per_core_ew.append(blocks)
    TB = 2 * halfmax        # tiles per block (even), window w gets tiles [w*TB/2,(w+1)*TB/2)
    T = NBLK * TB           # message tiles per core per layer
    assert T % 8 == 0

    per_core = []
    for c in range(NCORE):
        lo = c * NPC
        oh = np.zeros((K, T * 128), BF16)
        d1 = np.zeros((128, T * 64), BF16)
        dgd = np.zeros((128, NBLK * 128), BF16)
        gidx = np.zeros((128, T), np.int16)
        xg = np.zeros((128, T * 16), BF16)   # layer-1 h[src], edge-ordered
        for b in range(NBLK):
            for w in (0, 1):
                edges = per_core_ew[c][b][w]
                for t in range(TB // 2):
                    tau = b * TB + w * (TB // 2) + t
                    seg = edges[t * 128:(t + 1) * 128]
                    n = len(seg)
                    if n:
                        p = np.arange(n)
                        tt = etype[seg]
                        cols = tau * 128 + p
                        oh[tt, cols] = ea[seg, 0].astype(BF16)
                        oh[ET + tt, cols] = ea[seg, 1].astype(BF16)
                        if K == 3 * ET:
                            oh[2 * ET + tt, cols] = BF16(1.0)
                        d1[p, tau * 64 + (dst[seg] - lo - b * 128 - w * 64)] = BF16(1.0)
                    # gather indices (wrapped per 16 partitions within group g=tau%8)
                    g = tau % 8
                    q = tau // 8
                    j = np.arange(128)
                    srcs = np.zeros(128, np.int16)
                    srcs[:n] = src[seg].astype(np.int16)
                    gidx[16 * g + (j % 16), q * 8 + j // 16] = srcs
                    # layer-1 gathered x in final [edge, 16g+i] layout
                    xg[:, (q * 8 + g) * 16:(q * 8 + g) * 16 + 16] = \
                        x[srcs.astype(np.int64)].astype(BF16)
            pb = np.arange(128)
            dgd[pb, b * 128 + pb] = deg[lo + b * 128 + pb].astype(BF16)

        xl = np.zeros((128, NBLK * 16), BF16)
        ivd = np.zeros((128, NBLK), np.float32)
        g2 = np.zeros((128, NBLK * 2), BF16)
        for b in range(NBLK):
            nodes = lo + b * 128 + np.arange(128)
            xl[:, b * 16:(b + 1) * 16] = x[nodes].astype(BF16)
            ivd[:, b] = inv_deg[nodes]
            g2[:, b * 2 + (b >= 10)] = (ct[nodes] == 1).astype(np.float32).astype(BF16)
        cnt = np.array([[(ct[lo:lo + NPG] == 1).sum()], [(ct[lo + NPG:lo + NPC] == 1).sum()]], np.float32)
        ivc = 1.0 / np.maximum(cnt, 1.0)
        per_core.append({"onehotS": oh, "dst1h": d1, "dstdiag": dgd, "gidx": gidx,
                         "xgath": xg, "xloc": xl, "invdeg": ivd, "gate2": g2,
                         "invcnt": ivc})
    return TB, K, shared, per_core


_CACHE = {}


def _build(TB, K, debug_h=False):
    nocc = bool(os.environ.get("ATHENA_NOCC"))
    nogather = bool(os.environ.get("ATHENA_NOGATHER"))
    nowarm = bool(os.environ.get("ATHENA_NOWARM"))
    key = (TB, K, debug_h, nocc, nogather, nowarm)
    if key in _CACHE:
        return _CACHE[key]
    T = NBLK * TB
    NG = T // 8                       # 8-tile groups per layer
    dt = mybir.dt
    nc = bacc.Bacc("TRN2", target_bir_lowering=False, num_devices=NCORE)

    xT_d = nc.dram_tensor("xT", [16, N], dt.float32, kind="ExternalInput")
    xl_d = nc.dram_tensor("xloc", [128, NBLK * 16], dt.bfloat16, kind="ExternalInput")
    oh_d = nc.dram_tensor("onehotS", [K, T * 128], dt.bfloat16, kind="ExternalInput")
    d1_d = nc.dram_tensor("dst1h", [128, T * 64], dt.bfloat16, kind="ExternalInput")
    dg_d = nc.dram_tensor("dstdiag", [128, NBLK * 128], dt.bfloat16, kind="ExternalInput")
    gi_d = nc.dram_tensor("gidx", [128, T], dt.int16, kind="ExternalInput")
    xg_d = nc.dram_tensor("xgath", [128, T * 16], dt.bfloat16, kind="ExternalInput")
    ivd_d = nc.dram_tensor("invdeg", [128, NBLK], dt.float32, kind="ExternalInput")
    g2_d = nc.dram_tensor("gate2", [128, NBLK * 2], dt.bfloat16, kind="ExternalInput")
    ivc_d = nc.dram_tensor("invcnt", [2, 1], dt.float32, kind="ExternalInput")
    cw_d = nc.dram_tensor("clfw", [2, 64], dt.float32, kind="ExternalInput")
    cb_d = nc.dram_tensor("clfb", [2, 1], dt.float32, kind="ExternalInput")
    GT_d, rr_d, br_d = {}, {}, {}
    for l, (ci, co) in enumerate(LAYERS, 1):
        d = ci * co
        GT_d[l] = nc.dram_tensor(f"GT{l}", [K, d], dt.bfloat16, kind="ExternalInput")
        rr_d[l] = nc.dram_tensor(f"rootrep{l}", [128, d], dt.bfloat16, kind="ExternalInput")
        br_d[l] = nc.dram_tensor(f"biasrep{l}", [128, co], dt.float32, kind="ExternalInput")
    out_d = nc.dram_tensor("out", [2, 1], dt.float32, kind="ExternalOutput")
    hdbg_d = nc.dram_tensor("hdbg", [128, 4 * NBLK * 64], dt.float32,
                            kind="ExternalOutput") if debug_h else None
    hsh_d = nc.dram_tensor("hshard", [8, NPC], dt.bfloat16, kind="Internal")
    hfull_d = nc.dram_tensor("hfull", [NCORE * 8, NPC], dt.bfloat16, kind="Internal",
                             addr_space="Shared")
    r16_d = nc.dram_tensor("rep16", [16, 128], dt.float32, kind="ExternalInput")
    r64_d = nc.dram_tensor("rep64", [64, 8 * 128], dt.bfloat16, kind="ExternalInput")
    warm_in_d = nc.dram_tensor("warmin", [8, 16], dt.float32, kind="Internal")
    warm_out_d = nc.dram_tensor("warmout", [NCORE * 8, 16], dt.float32, kind="Internal",
                                addr_space="Shared")

    with tile.TileContext(nc) as tc:
        with tc.tile_pool(name="const", bufs=1) as cpool, \
             tc.tile_pool(name="stream", bufs=5) as spool, \
             tc.tile_pool(name="stg", bufs=1) as stgpool, \
             tc.tile_pool(name="work", bufs=4) as wpool, \
             tc.tile_pool(name="ps_s", bufs=4, space="PSUM") as ps_s, \
             tc.tile_pool(name="ps_agg", bufs=2, space="PSUM") as ps_agg, \
             tc.tile_pool(name="ps_tr", bufs=2, space="PSUM") as ps_tr:

            # ---- collective-stream warmup (overlaps layer-1 compute) ----
            if not nocc and not nowarm:
                wt = cpool.tile([8, 16], dt.float32)
                nc.vector.memset(wt[:], 0.0)
                nc.sync.dma_start(out=warm_in_d[:], in_=wt[:])
                nc.gpsimd.collective_compute(
                    kind="AllGather", op=mybir.AluOpType.bypass,
                    replica_groups=[list(range(NCORE))],
                    ins=[warm_in_d[:]], outs=[warm_out_d[:]])

            # ---- critical-path preamble ---------------------------------
            GT, rr, br = {}, {}, {}
            for l, (ci, co) in enumerate(LAYERS, 1):
                d = ci * co
                GT[l] = cpool.tile([K, d], dt.bfloat16, tag=f"GT{l}", name=f"GT{l}t")
                nc.sync.dma_start(out=GT[l][:], in_=GT_d[l][:])
            gi = cpool.tile([128, T], dt.int16)
            nc.sync.dma_start(out=gi[:], in_=gi_d[:])
            rep64 = cpool.tile([64, 8 * 128], dt.bfloat16)
            nc.sync.dma_start(out=rep64[:], in_=r64_d[:])
            hloc = cpool.tile([128, NBLK, 16], dt.bfloat16)
            nc.sync.dma_start(out=hloc[:], in_=xl_d[:].rearrange("p (b i) -> p b i", i=16))

            # gather table for layers 2-4 (rebuilt at every layer boundary)
            table = cpool.tile([128, N, 1], dt.float32)
            d1 = cpool.tile([128, T * 64], dt.bfloat16)
            NCH = 12
            d1_bounds = [(T * c // NCH) * 64 for c in range(NCH + 1)]
            d1_next = [0]

            def emit_d1_chunk():
                c = d1_next[0]
                if c < NCH:
                    nc.sync.dma_start(out=d1[:, d1_bounds[c]:d1_bounds[c + 1]],
                                      in_=d1_d[:, d1_bounds[c]:d1_bounds[c + 1]])
                    d1_next[0] = c + 1
            emit_d1_chunk()
            emit_d1_chunk()
            dg = cpool.tile([128, NBLK * 128], dt.bfloat16)
            nc.sync.dma_start(out=dg[:], in_=dg_d[:])
            ivd = cpool.tile([128, NBLK], dt.float32)
            nc.sync.dma_start(out=ivd[:], in_=ivd_d[:])
            for l in range(1, 5):
                d = LAYERS[l - 1][0] * LAYERS[l - 1][1]
                rr[l] = cpool.tile([128, d], dt.bfloat16, tag=f"rr{l}", name=f"rr{l}t")
                nc.sync.dma_start(out=rr[l][:], in_=rr_d[l][:])
                br[l] = cpool.tile([128, LAYERS[l - 1][1]], dt.float32, tag=f"br{l}",
                                   name=f"br{l}t")
                nc.sync.dma_start(out=br[l][:], in_=br_d[l][:])
            g2t = cpool.tile([128, NBLK * 2], dt.bfloat16)
            nc.sync.dma_start(out=g2t[:], in_=g2_d[:])
            h4 = cpool.tile([128, NBLK, 64], dt.bfloat16)
            ident = cpool.tile([128, 128], dt.float32)
            make_identity(nc, ident[:])
            identb = cpool.tile([128, 128], dt.bfloat16)
            make_identity(nc, identb[:])

            for l, (ci, co) in enumerate(LAYERS, 1):
                d = ci * co
                nts = max(1, 512 // d)        # message tiles per PSUM slot
                hdst = h4 if l == 4 else hloc
                # per-layer self-tiles: Vs_all[:, b, :] = rootrep * h_b (bcast over o)
                vsall = cpool.tile([128, NBLK, 512], dt.bfloat16, tag="vsall",
                                   name=f"vsall{l}")
                hb = hloc[:, 0, 0:ci]
                h_self = bass.AP(hb.tensor, hb.offset,
                                 [hb.ap[0], [16, NBLK], [0, co], [1, ci]])
                rr0 = rr[l][:]
                rr_b = bass.AP(rr0.tensor, rr0.offset,
                               [rr0.ap[0], [0, NBLK], [1, d]])
                nc.vector.tensor_tensor(
                    out=vsall[:, :, 0:d].rearrange("p b (o i) -> p b o i", i=ci),
                    in0=rr_b.rearrange("p b (o i) -> p b o i", i=ci),
                    in1=h_self, op=mybir.AluOpType.mult)

                # pre-gather h[src] with a rotating window (decouples the
                # gpsimd gather chain from the per-group pipeline); layer 1
                # instead streams host-precomputed x[src] straight from HBM
                GW = 11
                if l > 1:
                    htg_all = cpool.tile([128, GW * 128, 1], dt.float32,
                                         tag="htgall", name=f"htgall{l}")

                def emit_gather(qg):
                    if nogather or l == 1:
                        return
                    nc.gpsimd.ap_gather(
                        out_ap=htg_all[:, (qg % GW) * 128:(qg % GW + 1) * 128, :],
                        in_ap=table[:],
                        idxs_ap=gi[:, qg * 8:(qg + 1) * 8],
                        channels=128, num_elems=N, d=1, num_idxs=128)

                if nogather and l > 1:
                    nc.vector.memset(htg_all[:], 0.25)
                for q0 in range(min(GW, NG)):
                    emit_gather(q0)

                cur_agg = {}      # block -> (psum tile, [started_w0, started_w1])
                # prefetch one-hot chunks PF groups ahead, each split into two
                # row-half DMAs so transfers spread across DMA queues
                PF = 4
                KH = K // 2

                def emit_oh(qo):
                    t_ = spool.tile([K, 1024], dt.bfloat16, tag="oh",
                                    name=f"oh{l}_{qo}")
                    nc.sync.dma_start(out=t_[:],
                                      in_=oh_d[:, qo * 1024:(qo + 1) * 1024])
                    return t_

                def emit_xg(qo):
                    t_ = spool.tile([128, 128], dt.bfloat16, tag="xg",
                                    name=f"xg_{qo}")
                    nc.sync.dma_start(out=t_[:],
                                      in_=xg_d[:, qo * 128:(qo + 1) * 128])
                    return t_

                ohcs, xgs = {}, {}
                for q0 in range(min(PF, NG)):
                    ohcs[q0] = emit_oh(q0)
                    if l == 1:
                        xgs[q0] = emit_xg(q0)
                for q in range(NG):
                    ohc = ohcs.pop(q)
                    # A) all W matmuls of the group first (PE FIFO not blocked
                    # on the gather/transpose path)
                    Pss = []
                    for s in range(8 // nts):
                        Ps = ps_s.tile([128, 512], dt.float32, tag="s",
                                       name=f"s{l}_{q}_{s}")
                        for t in range(nts):
                            g8 = s * nts + t
                            nc.tensor.matmul(out=Ps[:, t * d:(t + 1) * d],
                                             lhsT=ohc[:, g8 * 128:(g8 + 1) * 128],
                                             rhs=GT[l][:], start=True, stop=True)
                        Pss.append(Ps)
                    if q + PF < NG:
                        ohcs[q + PF] = emit_oh(q + PF)
                        if l == 1:
                            xgs[q + PF] = emit_xg(q + PF)
                    if l == 1 and q % 4 == 1:
                        emit_d1_chunk()
                    qw = (q % GW) * 128
                    if l == 1:
                        htr = xgs.pop(q)
                    elif l == 4:
                        Ptr = ps_tr.tile([128, 128], dt.float32, tag="tr")
                        nc.tensor.transpose(out=Ptr[:],
                                            in_=htg_all[:, qw:qw + 128, 0],
                                            identity=ident[:])
                        htr = wpool.tile([128, 128], dt.bfloat16, tag="htr")
                        nc.scalar.copy(out=htr[:], in_=Ptr[:])
                    else:
                        htgb = wpool.tile([128, 128], dt.bfloat16, tag="htgb")
                        nc.scalar.copy(out=htgb[:],
                                       in_=htg_all[:, qw:qw + 128, 0])
                        Ptr = ps_tr.tile([128, 128], dt.bfloat16, tag="tr")
                        nc.tensor.transpose(out=Ptr[:], in_=htgb[:], identity=identb[:])
                        htr = wpool.tile([128, 128], dt.bfloat16, tag="htr")
                        nc.scalar.copy(out=htr[:], in_=Ptr[:])
                    if q + GW < NG:
                        emit_gather(q + GW)
                    # B) V = relu(A) * h[src]
                    Vss = []
                    for s in range(8 // nts):
                        g80 = s * nts
                        h_in1 = bass.AP(htr.tensor, htr[:].offset + g80 * 16,
                                        [htr[:].ap[0], [16, nts], [0, co], [1, ci]])
                        V = wpool.tile([128, 512], dt.bfloat16, tag="V",
                                       name=f"V{l}_{q}_{s}")
                        if l in (2, 3):
                            # h >= 0 here, so relu(A)*h == relu(A*h): multiply
                            # straight out of PSUM, then a cheap 4x-mode relu.
                            V2 = wpool.tile([128, 512], dt.bfloat16, tag="W",
                                            name=f"V2{l}_{q}_{s}")
                            nc.vector.tensor_tensor(
                                out=V2[:].rearrange("p (t o i) -> p t o i", t=nts, i=ci),
                                in0=Pss[s][:].rearrange("p (t o i) -> p t o i",
                                                        t=nts, i=ci),
                                in1=h_in1, op=mybir.AluOpType.mult)
                            nc.vector.tensor_scalar(out=V[:], in0=V2[:],
                                                    scalar1=0.0, scalar2=None,
                                                    op0=mybir.AluOpType.max)
                        else:
                            # relu-evac on ScalarE, mult on DVE
                            Wsl = wpool.tile([128, 512], dt.bfloat16, tag="W",
                                             name=f"W{l}_{q}_{s}")
                            nc.scalar.activation(
                                out=Wsl[:], in_=Pss[s][:],
                                func=mybir.ActivationFunctionType.Relu)
                            nc.vector.tensor_tensor(
                                out=V[:].rearrange("p (t o i) -> p t o i", t=nts, i=ci),
                                in0=Wsl[:].rearrange("p (t o i) -> p t o i",
                                                     t=nts, i=ci),
                                in1=h_in1, op=mybir.AluOpType.mult)
                        Vss.append(V)
                    # C) scatter-accumulate per tile
                    for g8 in range(8):
                        s, t = g8 // nts, g8 % nts
                        tau = 8 * q + g8
                        b = tau // TB
                        w = 0 if (tau - b * TB) < TB // 2 else 1
                        if b not in cur_agg:
                            Pagg_new = ps_agg.tile([128, 512], dt.float32,
                                                   tag="agg", name=f"agg{l}_{b}")
                            cur_agg[b] = (Pagg_new, [False, False])
                        Pagg, started = cur_agg[b]
                        nc.tensor.matmul(out=Pagg[w * 64:(w + 1) * 64, 0:d],
                                         lhsT=d1[:, tau * 64:(tau + 1) * 64],
                                         rhs=Vss[s][:, t * d:(t + 1) * d],
                                         start=not started[w], stop=False)
                        started[w] = True
                        if tau == b * TB + TB - 1:
                            # ---- block tail: finish node update -----
                            nc.tensor.matmul(out=Pagg[:, 0:d],
                                             lhsT=dg[:, b * 128:(b + 1) * 128],
                                             rhs=vsall[:, b, 0:d],
                                             start=False, stop=True)
                            S = wpool.tile([128, co], dt.float32, tag="S",
                                           name=f"S{l}_{b}")
                            nc.vector.tensor_reduce(
                                out=S[:],
                                in_=Pagg[:, 0:d].rearrange("p (o i) -> p o i", i=ci),
                                axis=mybir.AxisListType.X, op=mybir.AluOpType.add)
                            S2 = wpool.tile([128, co], dt.float32, tag="S2",
                                            name=f"S2{l}_{b}")
                            nc.scalar.activation(out=S2[:], in_=S[:],
                                                 func=mybir.ActivationFunctionType.Copy,
                                                 scale=ivd[:, b:b + 1])
                            S3 = wpool.tile([128, co], dt.float32, tag="S3",
                                            name=f"S3{l}_{b}")
                            nc.vector.tensor_tensor(out=S3[:], in0=S2[:],
                                                    in1=br[l][:],
                                                    op=mybir.AluOpType.add)
                            S4 = wpool.tile([128, co], dt.float32, tag="S4",
                                            name=f"S4{l}_{b}")
                            nc.vector.tensor_scalar(out=S4[:], in0=S3[:],
                                                    scalar1=0.0, scalar2=None,
                                                    op0=mybir.AluOpType.max)
                            nc.vector.tensor_copy(out=hdst[:, b, 0:co], in_=S4[:])
                            if debug_h:
                                nc.sync.dma_start(
                                    out=hdbg_d[:][:, ((l - 1) * NBLK + b) * 64:
                                                  ((l - 1) * NBLK + b) * 64 + co],
                                    in_=S4[:])
                            del cur_agg[b]

                # ---- share h across cores, refill gather table ----------
                if l < 4:
                    # deferred transposes h[:, b, :co] -> hTsb (off the PE
                    # critical path during the group loop)
                    for b in range(NBLK):
                        Ptr2 = ps_tr.tile([128, 128], dt.bfloat16, tag="tr",
                                          name=f"tr{l}_{b}")
                        nc.tensor.transpose(out=Ptr2[0:co, 0:128],
                                            in_=hdst[:, b, 0:co],
                                            identity=identb[:])
                        if b % 4 == 0:
                            hTs = wpool.tile([8, 512], dt.bfloat16, tag="hTs",
                                             name=f"hTs{l}_{b}")
                        nc.scalar.copy(out=hTs[:, (b % 4) * 128:(b % 4 + 1) * 128],
                                       in_=Ptr2[0:co, 0:128])
                        if b % 4 == 3:
                            nc.sync.dma_start(
                                out=hsh_d[:, (b - 3) * 128:(b + 1) * 128],
                                in_=hTs[:])
                    if not nocc:
                        nc.gpsimd.collective_compute(
                            kind="AllGather", op=mybir.AluOpType.bypass,
                            replica_groups=[list(range(NCORE))],
                            ins=[hsh_d[:]], outs=[hfull_d[:]])
                        # stage hfull into SBUF with many small DMAs (spread
                        # across queues), then rebuild the gather table with
                        # PE replication matmuls (DMA fabric here is slow)
                        hfs = stgpool.tile([64, NPC], dt.bfloat16, tag="stage",
                                           name=f"hfs{l}")
                        CQ = NPC // 2
                        for rg in range(4):
                            for cq in range(2):
                                nc.sync.dma_start(
                                    out=hfs[16 * rg:16 * rg + 16,
                                            cq * CQ:(cq + 1) * CQ],
                                    in_=hfull_d[16 * rg:16 * rg + 16,
                                                cq * CQ:(cq + 1) * CQ])
                        for c in range(8):
                            for sc in range(NPC // 512):
                                Pt = ps_s.tile([128, 512], dt.float32, tag="s",
                                               name=f"hrep{l}_{c}_{sc}")
                                nc.tensor.matmul(
                                    out=Pt[:], lhsT=rep64[:, c * 128:(c + 1) * 128],
                                    rhs=hfs[:, sc * 512:(sc + 1) * 512],
                                    start=True, stop=True)
                                dst = table[:, c * NPC + sc * 512:
                                            c * NPC + (sc + 1) * 512, 0]
                                if sc % 2 == 0:
                                    nc.vector.tensor_copy(out=dst, in_=Pt[:])
                                else:
                                    nc.scalar.copy(out=dst, in_=Pt[:])

            # ---- pooling + classifier -----------------------------------
            Pp = ps_tr.tile([128, 128], dt.float32, tag="tr")
            for b in range(NBLK):
                nc.tensor.matmul(out=Pp[0:2, 0:64], lhsT=g2t[:, b * 2:(b + 1) * 2],
                                 rhs=h4[:, b, :], start=(b == 0), stop=(b == NBLK - 1))
            pool = wpool.tile([2, 64], dt.float32, tag="pool")
            ivc = cpool.tile([2, 1], dt.float32)
            nc.sync.dma_start(out=ivc[:], in_=ivc_d[:])
            cw = cpool.tile([2, 64], dt.float32)
            nc.sync.dma_start(out=cw[:], in_=cw_d[:])
            cb = cpool.tile([2, 1], dt.float32)
            nc.sync.dma_start(out=cb[:], in_=cb_d[:])
            nc.vector.tensor_scalar(out=pool[:], in0=Pp[0:2, 0:64], scalar1=ivc[:],
                                    scalar2=None, op0=mybir.AluOpType.mult)
            pz = wpool.tile([2, 64], dt.float32, tag="pz")
            nc.vector.tensor_tensor(out=pz[:], in0=pool[:], in1=cw[:], op=mybir.AluOpType.mult)
            z = wpool.tile([2, 1], dt.float32, tag="z")
            nc.vector.tensor_reduce(out=z[:], in_=pz[:], axis=mybir.AxisListType.X,
                                    op=mybir.AluOpType.add)
            z2 = wpool.tile([2, 1], dt.float32, tag="z2")
            nc.vector.tensor_tensor(out=z2[:], in0=z[:], in1=cb[:], op=mybir.AluOpType.add)
            z3 = wpool.tile([2, 1], dt.float32, tag="z3")
            nc.scalar.activation(out=z3[:], in_=z2[:],
                                 func=mybir.ActivationFunctionType.Sigmoid)
            nc.sync.dma_start(out=out_d[:], in_=z3[:])

    nc.compile()
    _CACHE[key] = nc
    return nc


def kernel(**inputs):
    debug_h = bool(os.environ.get("ATHENA_DEBUG_H"))
    TB, K, shared, per_core = _prep(inputs)
    nc = _build(TB, K, debug_h)
    in_maps = []
    for c in range(NCORE):
        m = dict(shared)
        m.update(per_core[c])
        in_maps.append(m)
    res = run_bass_kernel_spmd(nc, in_maps, core_ids=list(range(NCORE)),
                               trace=bool(os.environ.get("ATHENA_TRACE")))
    kernel.last_results = res
    outs = [res.results[c]["out"] for c in range(NCORE)]
    return np.concatenate(outs, axis=0).astype(np.float32)
